# revision 2
# baseline (speedup 1.0000x reference)
"""Trainium2 Bass kernel for nn_BertAttention_78554951843978 (v3, q-major ctx).

Sharding: data-parallel over B (2 groups of 4 cores), tensor-parallel over
D within a group (256 dims = 4 heads per core).

Key structure:
  - hsT/weights in bf16; qT/kT bf16 (d-major, for scores)
  - probs fp8e5 from the exp activation; V fp8e4 in DoubleRow layout;
    ctx matmuls run fp8 DoubleRow TRANSPOSED: out[q, v-dim] with q on
    partitions, so the softmax denominator (ones column) and the
    layernorm A/B terms are all per-partition scalars -> no cross-
    partition broadcast bounces at all
  - x kept q-major [q % 128, qtile, D-slice]; residual projection done
    natural-layout (hs-stationary) straight into x
  - layernorm stats are DVE reduces (free-dim!) per query chunk,
    AllGather'ed per chunk (1.0x collective cost vs AllReduce's 1.875x),
    reduced locally, applied per chunk, pipelined behind the attention
  - PE warm-up spin at t=0 so the first real matmuls run at full clock
"""

import numpy as np
import ml_dtypes

import concourse.bass as bass
import concourse.tile as tile
from concourse import mybir
from concourse.bass_utils import run_bass_kernel_spmd

B, S, D, H = 2, 2048, 1024, 16
HD = 64
NCORES = 8
GROUPS = 4          # cores per batch
DC = D // GROUPS    # 256 dims per core
EPS = 1e-12
MP = 80             # V slot: 64 dims + 1 ones + pad to 16B slot stride

F32 = mybir.dt.float32
F32R = mybir.dt.float32r
BF16 = mybir.dt.bfloat16
F8E4 = mybir.dt.float8e4
F8E5 = mybir.dt.float8e5
AF = mybir.ActivationFunctionType
DR = mybir.MatmulPerfMode.DoubleRow
ALU = mybir.AluOpType

KT = D // 128    # 8 contraction tiles
MT = DC // 128   # 2 head pairs
NS = S // 512    # 4 query chunks of 512
ST = S // 128    # 16 key tiles
KP = ST // 2     # 8 key-tile pairs (DoubleRow)
QT = S // 128    # 16 query tiles of 128


def _split_waits(nc, keep=1):
    """Walrus rejects >1 sem wait per (non-EVSEM) instruction; hoist extras
    onto preceding single-wait NOPs on the same engine."""
    for bb in nc.main_func.blocks:
        insts = list(bb.instructions)
        out_list = []
        changed = False
        for inst in insts:
            si = inst.sync_info
            cap = 2 if isinstance(inst, mybir.InstEventSemaphore) else keep
            if si is not None and si.on_wait is not None and len(si.on_wait) > cap:
                waits = list(si.on_wait)
                for w in waits[cap:]:
                    out_list.append(mybir.InstNoOp(
                        name=nc.get_next_instruction_name(),
                        engine=inst.engine,
                        ins=[], outs=[],
                        sync_info=mybir.SyncInfo(on_wait=[w], on_update=[]),
                        bass_nofuse=True,
                    ))
                inst.sync_info = mybir.SyncInfo(
                    on_wait=waits[:cap], on_update=list(si.on_update or []))
                changed = True
            out_list.append(inst)
        if changed:
            bb.instructions = out_list


def build_bass():
    nc = bass.Bass(num_devices=NCORES)

    # ---------------- DRAM I/O ----------------
    hsT_d = nc.dram_tensor("hsT", [D, S], BF16, kind="ExternalInput")
    wqT_d = nc.dram_tensor("wqT", [D, DC], BF16, kind="ExternalInput")
    wkT_d = nc.dram_tensor("wkT", [D, DC], BF16, kind="ExternalInput")
    wvT_d = nc.dram_tensor("wvT", [D, DC], BF16, kind="ExternalInput")
    wpT_d = nc.dram_tensor("wpT", [D, DC], BF16, kind="ExternalInput")
    bq_d = nc.dram_tensor("bq", [DC], F32, kind="ExternalInput")
    bk_d = nc.dram_tensor("bk", [DC], F32, kind="ExternalInput")
    bv_d = nc.dram_tensor("bv", [DC], F32, kind="ExternalInput")
    gm_d = nc.dram_tensor("gamma", [DC], F32, kind="ExternalInput")
    bt_d = nc.dram_tensor("beta", [DC], F32, kind="ExternalInput")
    out_d = nc.dram_tensor("out", [S, DC], F32, kind="ExternalOutput")

    with tile.TileContext(nc) as tc:
        with (
            tc.tile_pool(name="persist", bufs=1) as persist,
            tc.tile_pool(name="dram", bufs=1, space="DRAM") as dram,
        ):
            # ------------- persistent SBUF -------------
            qT = persist.tile([128, MT, S], BF16)            # 8 KB/part
            kT = persist.tile([128, MT, S], BF16)
            x = persist.tile([128, QT, DC], F32)             # q-major, 16 KB
            # DoubleRow-packed aug V: [kp][parity][head g][MP]
            vA = persist.tile([128, KP, 2, GROUPS, MP], F8E4)
            # small constants: cols 0..3 = bq|bk (2 each), 4 = f32r ones,
            # 5 = eps, 6..261 = bv bcast, 262..517 gamma bcast, 518..773 beta
            cst = persist.tile([128, 6 + 3 * DC], F32)
            bq_s, bk_s = cst[:, 0:2], cst[:, 2:4]
            eps_c = cst[:, 5:6]
            bv_b = cst[:, 6:6 + DC]
            gm_b = cst[:, 6 + DC:6 + 2 * DC]
            bt_b = cst[:, 6 + 2 * DC:6 + 3 * DC]
            wsrc = persist.tile([128, 256], BF16)            # warm-up fodder
            onesr = persist.tile([128, 1], BF16)

            p1sb_cm = tc.tile_pool(name="p1sb", bufs=1)
            p1sb = p1sb_cm.__enter__()
            hsT = p1sb.tile([128, KT, S], BF16)              # 32 KB/part
            wq = p1sb.tile([128, KT, MT, 128], BF16)
            wk = p1sb.tile([128, KT, MT, 128], BF16)
            wv = p1sb.tile([128, KT, DC], BF16)
            wp = p1sb.tile([128, KT, DC], BF16)

            # input DMAs: few, large transfers (desc-gen is serialized);
            # first-needed slices first; scalar queue untouched.
            hsT_t = hsT_d.rearrange("(t p) s -> p t s", p=128)
            wq_t = wqT_d.rearrange("(t p) (m f) -> p t m f", p=128, f=128)
            wk_t = wkT_d.rearrange("(t p) (m f) -> p t m f", p=128, f=128)
            wv_t = wvT_d.rearrange("(t p) c -> p t c", p=128)
            wp_t = wpT_d.rearrange("(t p) c -> p t c", p=128)
            nc.sync.dma_start(out=wq, in_=wq_t)
            nc.gpsimd.dma_start(out=wk, in_=wk_t)
            for k in range(KT):
                e = nc.sync if k % 2 == 0 else nc.gpsimd
                e.dma_start(out=hsT[:, k, 0:512], in_=hsT_t[:, k, 0:512])
            nc.sync.dma_start(out=wv, in_=wv_t)
            for k in range(KT):
                e = nc.sync if k % 2 == 0 else nc.gpsimd
                e.dma_start(out=hsT[:, k, 512:1024], in_=hsT_t[:, k, 512:1024])
            nc.gpsimd.dma_start(out=wp, in_=wp_t)
            for k in range(KT):
                e = nc.sync if k % 2 == 0 else nc.gpsimd
                e.dma_start(out=hsT[:, k, 1024:2048],
                            in_=hsT_t[:, k, 1024:2048])
            nc.gpsimd.dma_start(out=bq_s, in_=bq_d.rearrange("(m p) -> p m", p=128))
            nc.gpsimd.dma_start(out=bk_s, in_=bk_d.rearrange("(m p) -> p m", p=128))
            nc.gpsimd.dma_start(out=bv_b, in_=bass.AP(
                tensor=bv_d[:].tensor, offset=0, ap=[[0, 128], [1, DC]]))
            nc.gpsimd.dma_start(out=gm_b, in_=bass.AP(
                tensor=gm_d[:].tensor, offset=0, ap=[[0, 128], [1, DC]]))
            nc.gpsimd.dma_start(out=bt_b, in_=bass.AP(
                tensor=bt_d[:].tensor, offset=0, ap=[[0, 128], [1, DC]]))
            nc.vector.memset(onesr, 1.0)
            nc.vector.memset(eps_c, EPS)
            nc.vector.memset(wsrc, 1.0)
            nc.vector.memset(vA, 0.0)
            nc.vector.memset(vA[:, :, :, :, HD:HD + 1], 1.0)

            with (
                tc.tile_pool(name="pps", bufs=2, space="PSUM") as pps,
                tc.tile_pool(name="scps", bufs=2, space="PSUM") as scps,
                tc.tile_pool(name="ctxps", bufs=2, space="PSUM") as ctxps,
                tc.tile_pool(name="ptp", bufs=3) as ptp,
                tc.tile_pool(name="small", bufs=2) as small,
                tc.tile_pool(name="stg", bufs=2) as stg,
                tc.tile_pool(name="xrp", bufs=2) as xrp,
                tc.tile_pool(name="rows", bufs=2) as rows,
            ):
                # PE warm-up: ~28 cheap matmuls so pe ramps to full clock
                # before the first real projection (which waits on DMAs).
                wps = pps.tile([128, 512], F32, name="gps")
                for i in range(28):
                    nc.tensor.matmul(out=wps[0:1, 0:256], lhsT=onesr,
                                     rhs=wsrc, start=True, stop=True)

                def proj_group(w_sb, m, n, bias, out_sb):
                    """One [128,512] block of a W-stationary projection."""
                    ps = pps.tile([128, 512], F32, name="gps")
                    for k in range(KT):
                        nc.tensor.matmul(
                            out=ps, lhsT=w_sb[:, k, m, :],
                            rhs=hsT[:, k, n * 512:(n + 1) * 512],
                            start=(k == 0), stop=(k == KT - 1))
                    o = out_sb[:, m, n * 512:(n + 1) * 512]
                    nc.vector.tensor_scalar_add(out=o, in0=ps, scalar1=bias)

                def v_group(j):
                    """V for key tile j -> fp8e4 DoubleRow slot, + bias."""
                    kp, par = divmod(j, 2)
                    ps = pps.tile([128, 512], F32, name="gps")
                    for k in range(KT):
                        nc.tensor.matmul(
                            out=ps[:, 0:DC],
                            lhsT=hsT[:, k, j * 128:(j + 1) * 128],
                            rhs=wv[:, k, :],
                            start=(k == 0), stop=(k == KT - 1))
                    nc.vector.tensor_add(
                        out=vA[:, kp, par, :, 0:HD],
                        in0=ps[:, 0:DC].rearrange("p (h d) -> p h d", d=HD),
                        in1=bv_b.rearrange("p (h d) -> p h d", d=HD))

                def r_group(qt):
                    """Residual hs@Wp.T for query tile qt, natural layout,
                    straight into q-major x."""
                    ps = pps.tile([128, 512], F32, name="gps")
                    for k in range(KT):
                        nc.tensor.matmul(
                            out=ps[:, 0:DC],
                            lhsT=hsT[:, k, qt * 128:(qt + 1) * 128],
                            rhs=wp[:, k, :],
                            start=(k == 0), stop=(k == KT - 1))
                    nc.vector.tensor_copy(out=x[:, qt, :], in_=ps[:, 0:DC])

                def g_q(m, n):
                    return lambda: proj_group(wq, m, n, bq_s[:, m:m + 1], qT)

                def g_k(m, n):
                    return lambda: proj_group(wk, m, n, bk_s[:, m:m + 1], kT)

                def g_r(qt):
                    return lambda: r_group(qt)

                for g in [g_q(0, 0), g_k(0, 0)]:
                    g()
                blk0 = {1: g_k(0, 1), 3: g_k(0, 2), 5: g_k(0, 3),
                        7: g_q(0, 1), 9: g_r(0), 11: g_r(1), 13: g_r(2),
                        15: g_r(3)}
                fillers = [
                    g_q(0, 2), g_r(4), g_r(5), g_r(6), g_r(7), g_q(0, 3),
                    g_k(1, 0), g_k(1, 1),
                    g_k(1, 2), g_k(1, 3), g_q(1, 0), g_r(8), g_r(9),
                    g_r(10), g_r(11), g_q(1, 1),
                    g_q(1, 2), g_q(1, 3), g_r(12), g_r(13), g_r(14),
                    g_r(15),
                ]
                fillers.reverse()   # consumed via pop()

                cc_in = dram.tile([NS, 1024], F32)
                cc_out = dram.tile([NS, GROUPS * 1024], F32)
                out_t = out_d.rearrange("(t p) c -> p t c", p=128)

                # ---------------- pipeline stages ----------------
                def s1a_xupdate(hp, qn, ctxc, den_r):
                    """x[q, head dims] += ctx * recip(denom); per-partition
                    scalars only. Deferred one block; pure DVE."""
                    for h2 in range(2):
                        g = 2 * hp + h2
                        for qt in range(4):
                            tmp = stg.tile([128, HD], F32, name="tmp")
                            nc.vector.tensor_scalar_mul(
                                out=tmp,
                                in0=ctxc[:, h2, qt * 128:qt * 128 + HD],
                                scalar1=den_r[:, h2, qt:qt + 1])
                            xs = x[:, 4 * qn + qt, g * HD:(g + 1) * HD]
                            nc.vector.tensor_tensor(
                                out=xs, in0=xs, in1=tmp, op=ALU.add)
                    if hp != MT - 1:
                        return None
                    xq = x[:, 4 * qn:4 * qn + 4, :]
                    xr = xrp.tile([128, 4, DC], F32, name="xr")
                    nc.vector.tensor_scalar_max(out=xr, in0=xq, scalar1=0.0)
                    x2 = xrp.tile([128, 4, DC], F32, name="x2")
                    nc.vector.tensor_tensor(
                        out=x2, in0=xr, in1=xr, op=ALU.mult)
                    return xr, x2

                def s1b_stats(qn, xr, x2):
                    """local stats via free-dim reduces + AllGather issue."""
                    st = rows.tile([128, 2, 4], F32, name="st")
                    nc.vector.tensor_reduce(
                        out=st[:, 0, :], in_=xr, axis=mybir.AxisListType.X,
                        op=ALU.add)
                    nc.vector.tensor_reduce(
                        out=st[:, 1, :], in_=x2, axis=mybir.AxisListType.X,
                        op=ALU.add)
                    # dram layout per rank: [kind v][q = qt*128 + qp]
                    nc.sync.dma_start(
                        out=bass.AP(tensor=cc_in.tensor,
                                    offset=cc_in.offset + qn * 1024,
                                    ap=[[1, 128], [512, 2], [128, 4]]),
                        in_=st)
                    nc.gpsimd.collective_compute(
                        "AllGather", ALU.bypass,
                        replica_groups=[[0, 1, 2, 3], [4, 5, 6, 7]],
                        ins=[cc_in[qn:qn + 1, :].opt()],
                        outs=[cc_out[qn:qn + 1, :].opt()],
                    )

                def s2_rowmath(qn):
                    """group stats -> per-query A (rstd), B (mean*rstd)."""
                    rsb = rows.tile([128, GROUPS, 2, 4], F32, name="rsb")
                    nc.sync.dma_start(
                        out=rsb,
                        in_=bass.AP(
                            tensor=cc_out.tensor,
                            offset=cc_out.offset + qn * GROUPS * 1024,
                            ap=[[1, 128], [1024, GROUPS], [512, 2],
                                [128, 4]]))
                    acc = rows.tile([128, 2, 4], F32, name="acc")
                    nc.vector.tensor_tensor(
                        out=acc, in0=rsb[:, 0], in1=rsb[:, 1], op=ALU.add)
                    nc.vector.tensor_tensor(
                        out=acc, in0=acc, in1=rsb[:, 2], op=ALU.add)
                    nc.vector.tensor_tensor(
                        out=acc, in0=acc, in1=rsb[:, 3], op=ALU.add)
                    mm = rows.tile([128, 2, 4], F32, name="mm")
                    nc.vector.tensor_scalar_mul(
                        out=mm, in0=acc, scalar1=1.0 / D)
                    var = rows.tile([128, 4], F32, name="var")
                    nc.vector.tensor_tensor(
                        out=var, in0=mm[:, 0, :], in1=mm[:, 0, :],
                        op=ALU.mult)
                    nc.vector.tensor_tensor(
                        out=var, in0=mm[:, 1, :], in1=var, op=ALU.subtract)
                    sd = rows.tile([128, 4], F32, name="sd")
                    nc.scalar.activation(
                        out=sd, in_=var, func=AF.Sqrt, bias=eps_c)
                    AB = rows.tile([128, 2, 4], F32, name="AB")
                    nc.vector.reciprocal(out=AB[:, 0, :], in_=sd)
                    nc.vector.tensor_tensor(
                        out=AB[:, 1, :], in0=mm[:, 0, :], in1=AB[:, 0, :],
                        op=ALU.mult)
                    return AB

                def s3_apply(qn, AB):
                    """out = (relu(x)*A - B)*gamma + beta, DMA out."""
                    for qt4 in range(4):
                        qt = 4 * qn + qt4
                        y = stg.tile([128, DC], F32, name="y")
                        nc.vector.tensor_scalar(
                            out=y, in0=x[:, qt, :],
                            scalar1=0.0, scalar2=AB[:, 0, qt4:qt4 + 1],
                            op0=ALU.max, op1=ALU.mult)
                        nc.vector.tensor_scalar(
                            out=y, in0=y, scalar1=AB[:, 1, qt4:qt4 + 1],
                            scalar2=None, op0=ALU.subtract)
                        nc.vector.tensor_tensor(
                            out=y, in0=y, in1=gm_b, op=ALU.mult)
                        nc.vector.tensor_tensor(
                            out=x[:, qt, :], in0=y, in1=bt_b, op=ALU.add)
                    nc.sync.dma_start(
                        out=out_t[:, 4 * qn:4 * qn + 4, :],
                        in_=x[:, 4 * qn:4 * qn + 4, :])

                pend_xu = []      # (hp, qn, ctxc, den_r)
                pend_st = []      # (qn, xr, x2)
                pend_ag = []      # (qn, issue_block)
                pend_s3 = []      # (qn, AB)
                bi = 0

                def do_s1a():
                    if pend_xu:
                        hp_, qn_, ctxc_, den_ = pend_xu.pop(0)
                        r = s1a_xupdate(hp_, qn_, ctxc_, den_)
                        if r is not None:
                            pend_st.append((qn_, r[0], r[1]))

                def do_s1b(bi):
                    if pend_st:
                        qn_, xr_, x2_ = pend_st.pop(0)
                        s1b_stats(qn_, xr_, x2_)
                        pend_ag.append((qn_, bi))

                def do_s2(bi, min_age=1):
                    if pend_ag and bi - pend_ag[0][1] >= min_age:
                        qn_, _ = pend_ag.pop(0)
                        pend_s3.append((qn_, s2_rowmath(qn_)))

                # ================= attention =================
                for hp in range(MT):
                    for qn in range(NS):
                        qs = slice(qn * 512, (qn + 1) * 512)
                        # one PSUM bank hosts 4 accumulation groups (one
                        # per query tile): matmul start=True zeroing is
                        # bank-granular, so pre-zero via DVE and accumulate
                        # with start=False throughout.
                        ctxA = ctxps.tile([128, 512], F32, name="ctx")
                        ctxB = ctxps.tile([128, 512], F32, name="ctx")
                        nc.vector.memset(ctxA, 0.0)
                        nc.vector.memset(ctxB, 0.0)

                        def ctx_mms(pt, kp):
                            for h2, cps in ((0, ctxA), (1, ctxB)):
                                for qt4 in range(4):
                                    nc.tensor.matmul(
                                        out=cps[:, qt4 * 128:qt4 * 128 + HD + 1],
                                        lhsT=pt[:, :, h2,
                                                qt4 * 128:(qt4 + 1) * 128],
                                        rhs=vA[:, kp, :, 2 * hp + h2,
                                               0:HD + 1],
                                        start=False, stop=(kp == KP - 1),
                                        perf_mode=DR)

                        prev = None
                        for kp in range(KP):
                            pt = ptp.tile([128, 2, 2, 512], F8E5, name="pt")
                            for par in range(2):
                                ks = 2 * kp + par
                                if hp == 0 and qn == 0:
                                    v_group(ks)
                                    if ks in blk0:
                                        blk0[ks]()
                                elif fillers and ks % 2 == 0:
                                    fillers.pop()()
                                sc = scps.tile([128, 1024], F32, name="sc")
                                kslc = slice(ks * 128, (ks + 1) * 128)
                                nc.tensor.matmul(
                                    out=sc[:, 0:512],
                                    lhsT=kT[0:64, hp, kslc],
                                    rhs=qT[0:64, hp, qs])
                                nc.tensor.matmul(
                                    out=sc[:, 512:1024],
                                    lhsT=kT[64:128, hp, kslc],
                                    rhs=qT[64:128, hp, qs])
                                nc.scalar.activation(
                                    out=pt[:, par], in_=sc, func=AF.Exp,
                                    scale=float(1.0 / np.sqrt(HD)))
                            if prev is not None:
                                ctx_mms(*prev)
                                if kp == 1:
                                    do_s1a()
                                elif kp == 4:
                                    do_s1b(bi)
                                elif kp == 6:
                                    do_s2(bi)
                            prev = (pt, kp)
                        ctx_mms(*prev)

                        # denominators (per-partition!) + ctx copy-out
                        den_r = small.tile([128, 2, 4], F32, name="den")
                        for h2, cps in ((0, ctxA), (1, ctxB)):
                            nc.vector.reciprocal(
                                out=den_r[:, h2, :],
                                in_=bass.AP(tensor=cps.tensor,
                                            offset=cps.offset + HD,
                                            ap=[list(cps.ap[0]), [128, 4]]))
                        ctxc = stg.tile([128, 2, 512], F32, name="ctxc")
                        nc.vector.tensor_copy(out=ctxc[:, 0, :], in_=ctxA)
                        nc.vector.tensor_copy(out=ctxc[:, 1, :], in_=ctxB)
                        pend_xu.append((hp, qn, ctxc, den_r))
                        if pend_s3:
                            s3_apply(*pend_s3.pop(0))
                        bi += 1

                # drain
                do_s1a()
                do_s1b(bi)
                do_s2(bi, min_age=0)
                if pend_s3:
                    s3_apply(*pend_s3.pop(0))
                do_s2(bi, min_age=0)
                if pend_s3:
                    s3_apply(*pend_s3.pop(0))
            p1sb_cm.__exit__(None, None, None)
    _split_waits(nc)
    return nc


_NC = None
LAST_RESULT = None


def _get_nc():
    global _NC
    if _NC is None:
        _NC = build_bass()
    return _NC


def kernel(hidden_states, Wq, bq, Wk, bk, Wv, bv, Wp, gamma, beta):
    hs = np.asarray(hidden_states, dtype=np.float32)
    Wq = np.asarray(Wq, np.float32)
    Wk = np.asarray(Wk, np.float32)
    Wv = np.asarray(Wv, np.float32)
    Wp = np.asarray(Wp, np.float32)
    bq = np.asarray(bq, np.float32)
    bk = np.asarray(bk, np.float32)
    bv = np.asarray(bv, np.float32)
    gamma = np.asarray(gamma, np.float32)
    beta = np.asarray(beta, np.float32)
    bf = ml_dtypes.bfloat16

    nc = _get_nc()
    in_maps = []
    for c in range(NCORES):
        b, g = divmod(c, GROUPS)
        sl = slice(g * DC, (g + 1) * DC)
        in_maps.append({
            "hsT": np.ascontiguousarray(hs[b].T.astype(bf)),
            "wqT": np.ascontiguousarray(Wq[sl].T.astype(bf)),
            "wkT": np.ascontiguousarray(Wk[sl].T.astype(bf)),
            "wvT": np.ascontiguousarray(Wv[sl].T.astype(bf)),
            "wpT": np.ascontiguousarray(Wp[sl].T.astype(bf)),
            "bq": np.ascontiguousarray(bq[sl]),
            "bk": np.ascontiguousarray(bk[sl]),
            "bv": np.ascontiguousarray(bv[sl]),
            "gamma": np.ascontiguousarray(gamma[sl]),
            "beta": np.ascontiguousarray(beta[sl]),
        })
    res = run_bass_kernel_spmd(nc, in_maps, core_ids=list(range(NCORES)))
    global LAST_RESULT
    LAST_RESULT = res
    out = np.empty((B, S, D), np.float32)
    for c, r in enumerate(res.results):
        b, g = divmod(c, GROUPS)
        out[b, :, g * DC:(g + 1) * DC] = r["out"]
    return out


# revision 3
# speedup vs baseline: 1.0090x; 1.0090x over previous
"""Trainium2 Bass kernel for nn_BertAttention_78554951843978 (v3, q-major ctx).

Sharding: data-parallel over B (2 groups of 4 cores), tensor-parallel over
D within a group (256 dims = 4 heads per core).

Key structure:
  - hsT/weights in bf16; qT/kT bf16 (d-major, for scores)
  - probs fp8e5 from the exp activation; V fp8e4 in DoubleRow layout;
    ctx matmuls run fp8 DoubleRow TRANSPOSED: out[q, v-dim] with q on
    partitions, so the softmax denominator (ones column) and the
    layernorm A/B terms are all per-partition scalars -> no cross-
    partition broadcast bounces at all
  - x kept q-major [q % 128, qtile, D-slice]; residual projection done
    natural-layout (hs-stationary) straight into x
  - layernorm stats are DVE reduces (free-dim!) per query chunk,
    AllGather'ed per chunk (1.0x collective cost vs AllReduce's 1.875x),
    reduced locally, applied per chunk, pipelined behind the attention
  - PE warm-up spin at t=0 so the first real matmuls run at full clock
"""

import numpy as np
import ml_dtypes

import concourse.bass as bass
import concourse.tile as tile
from concourse import mybir
from concourse.bass_utils import run_bass_kernel_spmd

B, S, D, H = 2, 2048, 1024, 16
HD = 64
NCORES = 8
GROUPS = 4          # cores per batch
DC = D // GROUPS    # 256 dims per core
EPS = 1e-12
MP = 80             # V slot: 64 dims + 1 ones + pad to 16B slot stride

F32 = mybir.dt.float32
F32R = mybir.dt.float32r
BF16 = mybir.dt.bfloat16
F8E4 = mybir.dt.float8e4
F8E5 = mybir.dt.float8e5
AF = mybir.ActivationFunctionType
DR = mybir.MatmulPerfMode.DoubleRow
ALU = mybir.AluOpType

KT = D // 128    # 8 contraction tiles
MT = DC // 128   # 2 head pairs
NS = S // 512    # 4 query chunks of 512
ST = S // 128    # 16 key tiles
KP = ST // 2     # 8 key-tile pairs (DoubleRow)
QT = S // 128    # 16 query tiles of 128


def _split_waits(nc, keep=1):
    """Walrus rejects >1 sem wait per (non-EVSEM) instruction; hoist extras
    onto preceding single-wait NOPs on the same engine."""
    for bb in nc.main_func.blocks:
        insts = list(bb.instructions)
        out_list = []
        changed = False
        for inst in insts:
            si = inst.sync_info
            cap = 2 if isinstance(inst, mybir.InstEventSemaphore) else keep
            if si is not None and si.on_wait is not None and len(si.on_wait) > cap:
                waits = list(si.on_wait)
                for w in waits[cap:]:
                    out_list.append(mybir.InstNoOp(
                        name=nc.get_next_instruction_name(),
                        engine=inst.engine,
                        ins=[], outs=[],
                        sync_info=mybir.SyncInfo(on_wait=[w], on_update=[]),
                        bass_nofuse=True,
                    ))
                inst.sync_info = mybir.SyncInfo(
                    on_wait=waits[:cap], on_update=list(si.on_update or []))
                changed = True
            out_list.append(inst)
        if changed:
            bb.instructions = out_list


def build_bass():
    nc = bass.Bass(num_devices=NCORES)

    # ---------------- DRAM I/O ----------------
    hsT_d = nc.dram_tensor("hsT", [D, S], BF16, kind="ExternalInput")
    wqT_d = nc.dram_tensor("wqT", [D, DC], BF16, kind="ExternalInput")
    wkT_d = nc.dram_tensor("wkT", [D, DC], BF16, kind="ExternalInput")
    wvT_d = nc.dram_tensor("wvT", [D, DC], BF16, kind="ExternalInput")
    wpT_d = nc.dram_tensor("wpT", [D, DC], BF16, kind="ExternalInput")
    bq_d = nc.dram_tensor("bq", [DC], F32, kind="ExternalInput")
    bk_d = nc.dram_tensor("bk", [DC], F32, kind="ExternalInput")
    bv_d = nc.dram_tensor("bv", [DC], F32, kind="ExternalInput")
    gm_d = nc.dram_tensor("gamma", [DC], F32, kind="ExternalInput")
    bt_d = nc.dram_tensor("beta", [DC], F32, kind="ExternalInput")
    out_d = nc.dram_tensor("out", [S, DC], F32, kind="ExternalOutput")

    with tile.TileContext(nc) as tc:
        with (
            tc.tile_pool(name="persist", bufs=1) as persist,
            tc.tile_pool(name="dram", bufs=1, space="DRAM") as dram,
        ):
            # ------------- persistent SBUF -------------
            qT = persist.tile([128, MT, S], BF16)            # 8 KB/part
            kT = persist.tile([128, MT, S], BF16)
            x = persist.tile([128, QT, DC], F32)             # q-major, 16 KB
            # DoubleRow-packed aug V: [kp][parity][head g][MP]
            vA = persist.tile([128, KP, 2, GROUPS, MP], F8E4)
            # small constants: cols 0..3 = bq|bk (2 each), 4 = f32r ones,
            # 5 = eps, 6..261 = bv bcast, 262..517 gamma bcast, 518..773 beta
            cst = persist.tile([128, 6 + 3 * DC], F32)
            bq_s, bk_s = cst[:, 0:2], cst[:, 2:4]
            eps_c = cst[:, 5:6]
            bv_b = cst[:, 6:6 + DC]
            gm_b = cst[:, 6 + DC:6 + 2 * DC]
            bt_b = cst[:, 6 + 2 * DC:6 + 3 * DC]
            wsrc = persist.tile([128, 256], BF16)            # warm-up fodder
            onesr = persist.tile([128, 1], BF16)

            p1sb_cm = tc.tile_pool(name="p1sb", bufs=1)
            p1sb = p1sb_cm.__enter__()
            hsT = p1sb.tile([128, KT, S], BF16)              # 32 KB/part
            wq = p1sb.tile([128, KT, MT, 128], BF16)
            wk = p1sb.tile([128, KT, MT, 128], BF16)
            wv = p1sb.tile([128, KT, DC], BF16)
            wp = p1sb.tile([128, KT, DC], BF16)

            # input DMAs: few, large transfers (desc-gen is serialized);
            # first-needed slices first; scalar queue untouched.
            hsT_t = hsT_d.rearrange("(t p) s -> p t s", p=128)
            wq_t = wqT_d.rearrange("(t p) (m f) -> p t m f", p=128, f=128)
            wk_t = wkT_d.rearrange("(t p) (m f) -> p t m f", p=128, f=128)
            wv_t = wvT_d.rearrange("(t p) c -> p t c", p=128)
            wp_t = wpT_d.rearrange("(t p) c -> p t c", p=128)
            nc.gpsimd.dma_start(out=bq_s, in_=bq_d.rearrange("(m p) -> p m", p=128))
            nc.gpsimd.dma_start(out=bk_s, in_=bk_d.rearrange("(m p) -> p m", p=128))
            nc.gpsimd.dma_start(out=bv_b, in_=bass.AP(
                tensor=bv_d[:].tensor, offset=0, ap=[[0, 128], [1, DC]]))
            nc.sync.dma_start(out=wq, in_=wq_t)
            for k in range(KT):
                e = nc.sync if k % 2 == 0 else nc.gpsimd
                e.dma_start(out=hsT[:, k, 0:512], in_=hsT_t[:, k, 0:512])
            nc.gpsimd.dma_start(out=wk, in_=wk_t)
            for k in range(KT):
                e = nc.sync if k % 2 == 0 else nc.gpsimd
                e.dma_start(out=hsT[:, k, 512:1024], in_=hsT_t[:, k, 512:1024])
            nc.sync.dma_start(out=wv, in_=wv_t)
            for k in range(KT):
                e = nc.sync if k % 2 == 0 else nc.gpsimd
                e.dma_start(out=hsT[:, k, 1024:2048],
                            in_=hsT_t[:, k, 1024:2048])
            nc.gpsimd.dma_start(out=wp, in_=wp_t)
            nc.gpsimd.dma_start(out=gm_b, in_=bass.AP(
                tensor=gm_d[:].tensor, offset=0, ap=[[0, 128], [1, DC]]))
            nc.gpsimd.dma_start(out=bt_b, in_=bass.AP(
                tensor=bt_d[:].tensor, offset=0, ap=[[0, 128], [1, DC]]))
            nc.vector.memset(onesr, 1.0)
            nc.vector.memset(eps_c, EPS)
            nc.vector.memset(wsrc, 1.0)
            nc.vector.memset(vA[:, :, :, :, HD:MP], 0.0)
            nc.vector.memset(vA[:, :, :, :, HD:HD + 1], 1.0)

            with (
                tc.tile_pool(name="pps", bufs=2, space="PSUM") as pps,
                tc.tile_pool(name="scps", bufs=2, space="PSUM") as scps,
                tc.tile_pool(name="ctxps", bufs=2, space="PSUM") as ctxps,
                tc.tile_pool(name="ptp", bufs=3) as ptp,
                tc.tile_pool(name="small", bufs=2) as small,
                tc.tile_pool(name="stg", bufs=2) as stg,
                tc.tile_pool(name="xrp", bufs=2) as xrp,
                tc.tile_pool(name="rows", bufs=2) as rows,
            ):
                # PE warm-up: ~28 cheap matmuls so pe ramps to full clock
                # before the first real projection (which waits on DMAs).
                wps = pps.tile([128, 512], F32, name="gps")
                for i in range(20):
                    nc.tensor.matmul(out=wps[0:1, 0:256], lhsT=onesr,
                                     rhs=wsrc, start=True, stop=True)

                def proj_group(w_sb, m, n, bias, out_sb):
                    """One [128,512] block of a W-stationary projection."""
                    ps = pps.tile([128, 512], F32, name="gps")
                    for k in range(KT):
                        nc.tensor.matmul(
                            out=ps, lhsT=w_sb[:, k, m, :],
                            rhs=hsT[:, k, n * 512:(n + 1) * 512],
                            start=(k == 0), stop=(k == KT - 1))
                    o = out_sb[:, m, n * 512:(n + 1) * 512]
                    nc.vector.tensor_scalar_add(out=o, in0=ps, scalar1=bias)

                def v_group(j):
                    """V for key tile j -> fp8e4 DoubleRow slot, + bias."""
                    kp, par = divmod(j, 2)
                    ps = pps.tile([128, 512], F32, name="gps")
                    for k in range(KT):
                        nc.tensor.matmul(
                            out=ps[:, 0:DC],
                            lhsT=hsT[:, k, j * 128:(j + 1) * 128],
                            rhs=wv[:, k, :],
                            start=(k == 0), stop=(k == KT - 1))
                    nc.vector.tensor_add(
                        out=vA[:, kp, par, :, 0:HD],
                        in0=ps[:, 0:DC].rearrange("p (h d) -> p h d", d=HD),
                        in1=bv_b.rearrange("p (h d) -> p h d", d=HD))

                def r_group(qt):
                    """Residual hs@Wp.T for query tile qt, natural layout,
                    straight into q-major x."""
                    ps = pps.tile([128, 512], F32, name="gps")
                    for k in range(KT):
                        nc.tensor.matmul(
                            out=ps[:, 0:DC],
                            lhsT=hsT[:, k, qt * 128:(qt + 1) * 128],
                            rhs=wp[:, k, :],
                            start=(k == 0), stop=(k == KT - 1))
                    nc.vector.tensor_copy(out=x[:, qt, :], in_=ps[:, 0:DC])

                def g_q(m, n):
                    return lambda: proj_group(wq, m, n, bq_s[:, m:m + 1], qT)

                def g_k(m, n):
                    return lambda: proj_group(wk, m, n, bk_s[:, m:m + 1], kT)

                def g_r(qt):
                    return lambda: r_group(qt)

                for g in [g_q(0, 0), g_k(0, 0)]:
                    g()
                blk0 = {1: g_k(0, 1), 3: g_k(0, 2), 5: g_k(0, 3),
                        7: g_q(0, 1), 9: g_r(0), 11: g_r(1), 13: g_r(2),
                        15: g_r(3)}
                fillers = [
                    g_q(0, 2), g_r(4), g_r(5), g_r(6), g_r(7), g_q(0, 3),
                    g_k(1, 0), g_k(1, 1),
                    g_k(1, 2), g_k(1, 3), g_q(1, 0), g_r(8), g_r(9),
                    g_r(10), g_r(11), g_q(1, 1),
                    g_q(1, 2), g_q(1, 3), g_r(12), g_r(13), g_r(14),
                    g_r(15),
                ]
                fillers.reverse()   # consumed via pop()

                cc_in = dram.tile([NS, 1024], F32)
                cc_out = dram.tile([NS, GROUPS * 1024], F32)
                out_t = out_d.rearrange("(t p) c -> p t c", p=128)

                # ---------------- pipeline stages ----------------
                def s1a_xupdate(hp, qn, ctxc, den_r):
                    """x[q, head dims] += ctx * recip(denom); per-partition
                    scalars only. Deferred one block; pure DVE."""
                    for h2 in range(2):
                        g = 2 * hp + h2
                        for qt in range(4):
                            tmp = stg.tile([128, HD], F32, name="tmp")
                            nc.vector.tensor_scalar_mul(
                                out=tmp,
                                in0=ctxc[:, h2, qt * 128:qt * 128 + HD],
                                scalar1=den_r[:, h2, qt:qt + 1])
                            xs = x[:, 4 * qn + qt, g * HD:(g + 1) * HD]
                            nc.vector.tensor_tensor(
                                out=xs, in0=xs, in1=tmp, op=ALU.add)
                    if hp != MT - 1:
                        return None
                    xq = x[:, 4 * qn:4 * qn + 4, :]
                    xr = xrp.tile([128, 4, DC], F32, name="xr")
                    nc.vector.tensor_scalar_max(out=xr, in0=xq, scalar1=0.0)
                    x2 = xrp.tile([128, 4, DC], F32, name="x2")
                    nc.vector.tensor_tensor(
                        out=x2, in0=xr, in1=xr, op=ALU.mult)
                    return xr, x2

                def s1b_stats(qn, xr, x2):
                    """local stats via free-dim reduces + AllGather issue."""
                    st = rows.tile([128, 2, 4], F32, name="st")
                    nc.vector.tensor_reduce(
                        out=st[:, 0, :], in_=xr, axis=mybir.AxisListType.X,
                        op=ALU.add)
                    nc.vector.tensor_reduce(
                        out=st[:, 1, :], in_=x2, axis=mybir.AxisListType.X,
                        op=ALU.add)
                    # dram layout per rank: [kind v][q = qt*128 + qp]
                    nc.sync.dma_start(
                        out=bass.AP(tensor=cc_in.tensor,
                                    offset=cc_in.offset + qn * 1024,
                                    ap=[[1, 128], [512, 2], [128, 4]]),
                        in_=st)
                    nc.gpsimd.collective_compute(
                        "AllGather", ALU.bypass,
                        replica_groups=[[0, 1, 2, 3], [4, 5, 6, 7]],
                        ins=[cc_in[qn:qn + 1, :].opt()],
                        outs=[cc_out[qn:qn + 1, :].opt()],
                    )

                def s2_rowmath(qn, dmae=None):
                    """group stats -> per-query A (rstd), B (mean*rstd)."""
                    dmae = dmae or nc.sync
                    rsb = rows.tile([128, GROUPS, 2, 4], F32, name="rsb")
                    dmae.dma_start(
                        out=rsb,
                        in_=bass.AP(
                            tensor=cc_out.tensor,
                            offset=cc_out.offset + qn * GROUPS * 1024,
                            ap=[[1, 128], [1024, GROUPS], [512, 2],
                                [128, 4]]))
                    acc = rows.tile([128, 2, 4], F32, name="acc")
                    nc.vector.tensor_tensor(
                        out=acc, in0=rsb[:, 0], in1=rsb[:, 1], op=ALU.add)
                    nc.vector.tensor_tensor(
                        out=acc, in0=acc, in1=rsb[:, 2], op=ALU.add)
                    nc.vector.tensor_tensor(
                        out=acc, in0=acc, in1=rsb[:, 3], op=ALU.add)
                    mm = rows.tile([128, 2, 4], F32, name="mm")
                    nc.vector.tensor_scalar_mul(
                        out=mm, in0=acc, scalar1=1.0 / D)
                    var = rows.tile([128, 4], F32, name="var")
                    nc.vector.tensor_tensor(
                        out=var, in0=mm[:, 0, :], in1=mm[:, 0, :],
                        op=ALU.mult)
                    nc.vector.tensor_tensor(
                        out=var, in0=mm[:, 1, :], in1=var, op=ALU.subtract)
                    sd = rows.tile([128, 4], F32, name="sd")
                    nc.scalar.activation(
                        out=sd, in_=var, func=AF.Sqrt, bias=eps_c)
                    AB = rows.tile([128, 2, 4], F32, name="AB")
                    nc.vector.reciprocal(out=AB[:, 0, :], in_=sd)
                    nc.vector.tensor_tensor(
                        out=AB[:, 1, :], in0=mm[:, 0, :], in1=AB[:, 0, :],
                        op=ALU.mult)
                    return AB

                def s3_apply(qn, AB, dmae=None, pool=False):
                    """out = (relu(x)*A - B)*gamma + beta, DMA out."""
                    dmae = dmae or nc.sync
                    for qt4 in range(4):
                        qt = 4 * qn + qt4
                        eng = nc.gpsimd if (pool and qt4 % 2) else nc.vector
                        y = stg.tile([128, DC], F32, name="y")
                        eng.tensor_scalar(
                            out=y, in0=x[:, qt, :],
                            scalar1=0.0, scalar2=AB[:, 0, qt4:qt4 + 1],
                            op0=ALU.max, op1=ALU.mult)
                        eng.tensor_scalar(
                            out=y, in0=y, scalar1=AB[:, 1, qt4:qt4 + 1],
                            scalar2=None, op0=ALU.subtract)
                        eng.tensor_tensor(
                            out=y, in0=y, in1=gm_b, op=ALU.mult)
                        eng.tensor_tensor(
                            out=x[:, qt, :], in0=y, in1=bt_b, op=ALU.add)
                    dmae.dma_start(
                        out=out_t[:, 4 * qn:4 * qn + 4, :],
                        in_=x[:, 4 * qn:4 * qn + 4, :])

                pend_xu = []      # (hp, qn, ctxc, den_r)
                pend_st = []      # (qn, xr, x2)
                pend_ag = []      # (qn, issue_block)
                pend_s3 = []      # (qn, AB)
                bi = 0

                def do_s1a():
                    if pend_xu:
                        hp_, qn_, ctxc_, den_ = pend_xu.pop(0)
                        r = s1a_xupdate(hp_, qn_, ctxc_, den_)
                        if r is not None:
                            pend_st.append((qn_, r[0], r[1]))

                def do_s1b(bi):
                    if pend_st:
                        qn_, xr_, x2_ = pend_st.pop(0)
                        s1b_stats(qn_, xr_, x2_)
                        pend_ag.append((qn_, bi))

                def do_s2(bi, min_age=1, dmae=None):
                    if pend_ag and bi - pend_ag[0][1] >= min_age:
                        qn_, _ = pend_ag.pop(0)
                        pend_s3.append((qn_, s2_rowmath(qn_, dmae)))

                # ================= attention =================
                BLOCKS = [(0, 0), (0, 1), (0, 2), (0, 3),
                          (1, 0), (1, 1), (1, 2), (1, 3)]
                if True:
                    for hp, qn in BLOCKS:
                        qs = slice(qn * 512, (qn + 1) * 512)
                        # one PSUM bank hosts 4 accumulation groups (one
                        # per query tile): matmul start=True zeroing is
                        # bank-granular, so pre-zero via DVE and accumulate
                        # with start=False throughout.
                        ctxA = ctxps.tile([128, 512], F32, name="ctx")
                        ctxB = ctxps.tile([128, 512], F32, name="ctx")
                        nc.vector.memset(ctxA, 0.0)
                        nc.vector.memset(ctxB, 0.0)

                        def ctx_mms(pt, kp):
                            for h2, cps in ((0, ctxA), (1, ctxB)):
                                for qt4 in range(4):
                                    nc.tensor.matmul(
                                        out=cps[:, qt4 * 128:qt4 * 128 + HD + 1],
                                        lhsT=pt[:, :, h2,
                                                qt4 * 128:(qt4 + 1) * 128],
                                        rhs=vA[:, kp, :, 2 * hp + h2,
                                               0:HD + 1],
                                        start=False, stop=(kp == KP - 1),
                                        perf_mode=DR)

                        prev = None
                        for kp in range(KP):
                            pt = ptp.tile([128, 2, 2, 512], F8E5, name="pt")
                            for par in range(2):
                                ks = 2 * kp + par
                                sc = scps.tile([128, 1024], F32, name="sc")
                                kslc = slice(ks * 128, (ks + 1) * 128)
                                nc.tensor.matmul(
                                    out=sc[:, 0:512],
                                    lhsT=kT[0:64, hp, kslc],
                                    rhs=qT[0:64, hp, qs])
                                nc.tensor.matmul(
                                    out=sc[:, 512:1024],
                                    lhsT=kT[64:128, hp, kslc],
                                    rhs=qT[64:128, hp, qs])
                                nc.scalar.activation(
                                    out=pt[:, par], in_=sc, func=AF.Exp,
                                    scale=float(1.0 / np.sqrt(HD)))
                                if hp == 0 and qn == 0:
                                    v_group(ks)
                                    if ks in blk0:
                                        blk0[ks]()
                                elif fillers and ks % 2 == 0:
                                    fillers.pop()()
                            if prev is not None:
                                ctx_mms(*prev)
                                if kp == 1:
                                    do_s1a()
                                elif kp == 3:
                                    do_s1b(bi)
                                elif kp == 5:
                                    do_s2(bi)
                            prev = (pt, kp)
                        ctx_mms(*prev)

                        # denominators (per-partition!) + ctx copy-out
                        den_r = small.tile([128, 2, 4], F32, name="den")
                        for h2, cps in ((0, ctxA), (1, ctxB)):
                            nc.vector.reciprocal(
                                out=den_r[:, h2, :],
                                in_=bass.AP(tensor=cps.tensor,
                                            offset=cps.offset + HD,
                                            ap=[list(cps.ap[0]), [128, 4]]))
                        ctxc = stg.tile([128, 2, 512], F32, name="ctxc")
                        nc.vector.tensor_copy(out=ctxc[:, 0, :], in_=ctxA)
                        nc.vector.tensor_copy(out=ctxc[:, 1, :], in_=ctxB)
                        pend_xu.append((hp, qn, ctxc, den_r))
                        if pend_s3:
                            s3_apply(*pend_s3.pop(0))
                        bi += 1

                # drain: qn=3's rowmath DMA rides the idle scalar queue so
                # it isn't stuck behind qn=2's apply/output traffic
                do_s1a()
                do_s1b(bi)
                do_s2(bi, min_age=0)                   # qn=2
                do_s2(bi, min_age=0, dmae=nc.scalar)   # qn=3 (waits AG(3))
                s3_apply(*pend_s3.pop(0), pool=True)   # qn=2
                s3_apply(*pend_s3.pop(0), dmae=nc.scalar, pool=True)
            p1sb_cm.__exit__(None, None, None)
    _split_waits(nc)
    return nc


_NC = None
LAST_RESULT = None


def _get_nc():
    global _NC
    if _NC is None:
        _NC = build_bass()
    return _NC


def kernel(hidden_states, Wq, bq, Wk, bk, Wv, bv, Wp, gamma, beta):
    hs = np.asarray(hidden_states, dtype=np.float32)
    Wq = np.asarray(Wq, np.float32)
    Wk = np.asarray(Wk, np.float32)
    Wv = np.asarray(Wv, np.float32)
    Wp = np.asarray(Wp, np.float32)
    bq = np.asarray(bq, np.float32)
    bk = np.asarray(bk, np.float32)
    bv = np.asarray(bv, np.float32)
    gamma = np.asarray(gamma, np.float32)
    beta = np.asarray(beta, np.float32)
    bf = ml_dtypes.bfloat16

    nc = _get_nc()
    in_maps = []
    for c in range(NCORES):
        b, g = divmod(c, GROUPS)
        sl = slice(g * DC, (g + 1) * DC)
        in_maps.append({
            "hsT": np.ascontiguousarray(hs[b].T.astype(bf)),
            "wqT": np.ascontiguousarray(Wq[sl].T.astype(bf)),
            "wkT": np.ascontiguousarray(Wk[sl].T.astype(bf)),
            "wvT": np.ascontiguousarray(Wv[sl].T.astype(bf)),
            "wpT": np.ascontiguousarray(Wp[sl].T.astype(bf)),
            "bq": np.ascontiguousarray(bq[sl]),
            "bk": np.ascontiguousarray(bk[sl]),
            "bv": np.ascontiguousarray(bv[sl]),
            "gamma": np.ascontiguousarray(gamma[sl]),
            "beta": np.ascontiguousarray(beta[sl]),
        })
    res = run_bass_kernel_spmd(nc, in_maps, core_ids=list(range(NCORES)))
    global LAST_RESULT
    LAST_RESULT = res
    out = np.empty((B, S, D), np.float32)
    for c, r in enumerate(res.results):
        b, g = divmod(c, GROUPS)
        out[b, :, g * DC:(g + 1) * DC] = r["out"]
    return out


# revision 5
# speedup vs baseline: 1.0134x; 1.0044x over previous
"""Trainium2 Bass kernel for nn_BertAttention_78554951843978 (v3, q-major ctx).

Sharding: data-parallel over B (2 groups of 4 cores), tensor-parallel over
D within a group (256 dims = 4 heads per core).

Key structure:
  - hsT/weights in bf16; qT/kT bf16 (d-major, for scores)
  - probs fp8e5 from the exp activation; V fp8e4 in DoubleRow layout;
    ctx matmuls run fp8 DoubleRow TRANSPOSED: out[q, v-dim] with q on
    partitions, so the softmax denominator (ones column) and the
    layernorm A/B terms are all per-partition scalars -> no cross-
    partition broadcast bounces at all
  - x kept q-major [q % 128, qtile, D-slice]; residual projection done
    natural-layout (hs-stationary) straight into x
  - layernorm stats are DVE reduces (free-dim!) per query chunk,
    AllGather'ed per chunk (1.0x collective cost vs AllReduce's 1.875x),
    reduced locally, applied per chunk, pipelined behind the attention
  - PE warm-up spin at t=0 so the first real matmuls run at full clock
"""

import numpy as np
import ml_dtypes

import concourse.bass as bass
import concourse.tile as tile
from concourse import mybir
from concourse.bass_utils import run_bass_kernel_spmd

B, S, D, H = 2, 2048, 1024, 16
HD = 64
NCORES = 8
GROUPS = 4          # cores per batch
DC = D // GROUPS    # 256 dims per core
EPS = 1e-12
MP = 80             # V slot: 64 dims + 1 ones + pad to 16B slot stride

F32 = mybir.dt.float32
F32R = mybir.dt.float32r
BF16 = mybir.dt.bfloat16
F8E4 = mybir.dt.float8e4
F8E5 = mybir.dt.float8e5
AF = mybir.ActivationFunctionType
DR = mybir.MatmulPerfMode.DoubleRow
ALU = mybir.AluOpType

KT = D // 128    # 8 contraction tiles
MT = DC // 128   # 2 head pairs
NS = S // 512    # 4 query chunks of 512
ST = S // 128    # 16 key tiles
KP = ST // 2     # 8 key-tile pairs (DoubleRow)
QT = S // 128    # 16 query tiles of 128


def _split_waits(nc, keep=1):
    """Walrus rejects >1 sem wait per (non-EVSEM) instruction; hoist extras
    onto preceding single-wait NOPs on the same engine."""
    for bb in nc.main_func.blocks:
        insts = list(bb.instructions)
        out_list = []
        changed = False
        for inst in insts:
            si = inst.sync_info
            cap = 2 if isinstance(inst, mybir.InstEventSemaphore) else keep
            if si is not None and si.on_wait is not None and len(si.on_wait) > cap:
                waits = list(si.on_wait)
                for w in waits[cap:]:
                    out_list.append(mybir.InstNoOp(
                        name=nc.get_next_instruction_name(),
                        engine=inst.engine,
                        ins=[], outs=[],
                        sync_info=mybir.SyncInfo(on_wait=[w], on_update=[]),
                        bass_nofuse=True,
                    ))
                inst.sync_info = mybir.SyncInfo(
                    on_wait=waits[:cap], on_update=list(si.on_update or []))
                changed = True
            out_list.append(inst)
        if changed:
            bb.instructions = out_list


def build_bass():
    nc = bass.Bass(num_devices=NCORES)

    # ---------------- DRAM I/O ----------------
    hsT_d = nc.dram_tensor("hsT", [D, S], BF16, kind="ExternalInput")
    wqT_d = nc.dram_tensor("wqT", [D, DC], BF16, kind="ExternalInput")
    wkT_d = nc.dram_tensor("wkT", [D, DC], BF16, kind="ExternalInput")
    wvT_d = nc.dram_tensor("wvT", [D, DC], BF16, kind="ExternalInput")
    wpT_d = nc.dram_tensor("wpT", [D, DC], BF16, kind="ExternalInput")
    bq_d = nc.dram_tensor("bq", [DC], F32, kind="ExternalInput")
    bk_d = nc.dram_tensor("bk", [DC], F32, kind="ExternalInput")
    bv_d = nc.dram_tensor("bv", [DC], F32, kind="ExternalInput")
    gm_d = nc.dram_tensor("gamma", [DC], F32, kind="ExternalInput")
    bt_d = nc.dram_tensor("beta", [DC], F32, kind="ExternalInput")
    out_d = nc.dram_tensor("out", [S, DC], F32, kind="ExternalOutput")

    with tile.TileContext(nc) as tc:
        with (
            tc.tile_pool(name="persist", bufs=1) as persist,
            tc.tile_pool(name="dram", bufs=1, space="DRAM") as dram,
        ):
            # ------------- persistent SBUF -------------
            qT = persist.tile([128, MT, S], BF16)            # 8 KB/part
            kT = persist.tile([128, MT, S], BF16)
            x = persist.tile([128, QT, DC], F32)             # q-major, 16 KB
            # DoubleRow-packed aug V: [kp][parity][head g][MP]
            vA = persist.tile([128, KP, 2, GROUPS, MP], F8E4)
            # small constants: cols 0..3 = bq|bk (2 each), 4 = f32r ones,
            # 5 = eps, 6..261 = bv bcast, 262..517 gamma bcast, 518..773 beta
            cst = persist.tile([128, 6 + 3 * DC], F32)
            bq_s, bk_s = cst[:, 0:2], cst[:, 2:4]
            eps_c = cst[:, 5:6]
            bv_b = cst[:, 6:6 + DC]
            gm_b = cst[:, 6 + DC:6 + 2 * DC]
            bt_b = cst[:, 6 + 2 * DC:6 + 3 * DC]
            wsrc = persist.tile([128, 256], BF16)            # warm-up fodder
            onesr = persist.tile([128, 1], BF16)

            p1sb_cm = tc.tile_pool(name="p1sb", bufs=1)
            p1sb = p1sb_cm.__enter__()
            hsT = p1sb.tile([128, KT, S], BF16)              # 32 KB/part
            wq = p1sb.tile([128, KT, MT, 128], BF16)
            wk = p1sb.tile([128, KT, MT, 128], BF16)
            wv = p1sb.tile([128, KT, DC], BF16)
            wp = p1sb.tile([128, KT, DC], BF16)

            # input DMAs: few, large transfers (desc-gen is serialized);
            # first-needed slices first; scalar queue untouched.
            hsT_t = hsT_d.rearrange("(t p) s -> p t s", p=128)
            wq_t = wqT_d.rearrange("(t p) (m f) -> p t m f", p=128, f=128)
            wk_t = wkT_d.rearrange("(t p) (m f) -> p t m f", p=128, f=128)
            wv_t = wvT_d.rearrange("(t p) c -> p t c", p=128)
            wp_t = wpT_d.rearrange("(t p) c -> p t c", p=128)
            nc.gpsimd.dma_start(out=bq_s, in_=bq_d.rearrange("(m p) -> p m", p=128))
            nc.gpsimd.dma_start(out=bk_s, in_=bk_d.rearrange("(m p) -> p m", p=128))
            nc.sync.dma_start(out=wq, in_=wq_t)
            for k in range(KT):
                e = nc.sync if k % 2 == 0 else nc.gpsimd
                e.dma_start(out=hsT[:, k, 0:512], in_=hsT_t[:, k, 0:512])
            nc.gpsimd.dma_start(out=wk, in_=wk_t)
            nc.gpsimd.dma_start(out=bv_b, in_=bass.AP(
                tensor=bv_d[:].tensor, offset=0, ap=[[0, 128], [1, DC]]))
            nc.sync.dma_start(out=wv, in_=wv_t)
            for k in range(KT):
                e = nc.sync if k % 2 == 0 else nc.gpsimd
                e.dma_start(out=hsT[:, k, 512:1024], in_=hsT_t[:, k, 512:1024])
            for k in range(KT):
                e = nc.sync if k % 2 == 0 else nc.gpsimd
                e.dma_start(out=hsT[:, k, 1024:2048],
                            in_=hsT_t[:, k, 1024:2048])
            nc.gpsimd.dma_start(out=wp, in_=wp_t)
            nc.gpsimd.dma_start(out=gm_b, in_=bass.AP(
                tensor=gm_d[:].tensor, offset=0, ap=[[0, 128], [1, DC]]))
            nc.gpsimd.dma_start(out=bt_b, in_=bass.AP(
                tensor=bt_d[:].tensor, offset=0, ap=[[0, 128], [1, DC]]))
            nc.vector.memset(onesr, 1.0)
            nc.vector.memset(eps_c, EPS)
            nc.vector.memset(wsrc, 1.0)
            nc.vector.memset(vA[:, :, :, :, HD:MP], 0.0)
            nc.vector.memset(vA[:, :, :, :, HD:HD + 1], 1.0)

            with (
                tc.tile_pool(name="pps", bufs=2, space="PSUM") as pps,
                tc.tile_pool(name="scps", bufs=2, space="PSUM") as scps,
                tc.tile_pool(name="ctxps", bufs=2, space="PSUM") as ctxps,
                tc.tile_pool(name="ptp", bufs=4) as ptp,
                tc.tile_pool(name="small", bufs=2) as small,
                tc.tile_pool(name="stg", bufs=2) as stg,
                tc.tile_pool(name="xrp", bufs=2) as xrp,
                tc.tile_pool(name="rows", bufs=2) as rows,
            ):
                # PE warm-up: ~28 cheap matmuls so pe ramps to full clock
                # before the first real projection (which waits on DMAs).
                wps = pps.tile([128, 512], F32, name="gps")
                for i in range(32):
                    nc.tensor.matmul(out=wps[0:1, 0:256], lhsT=onesr,
                                     rhs=wsrc, start=True, stop=True)

                def proj_group(w_sb, m, n, bias, out_sb):
                    """One [128,512] block of a W-stationary projection."""
                    ps = pps.tile([128, 512], F32, name="gps")
                    for k in range(KT):
                        nc.tensor.matmul(
                            out=ps, lhsT=w_sb[:, k, m, :],
                            rhs=hsT[:, k, n * 512:(n + 1) * 512],
                            start=(k == 0), stop=(k == KT - 1))
                    o = out_sb[:, m, n * 512:(n + 1) * 512]
                    nc.vector.tensor_scalar_add(out=o, in0=ps, scalar1=bias)

                def v_group(j):
                    """V for key tile j -> fp8e4 DoubleRow slot, + bias."""
                    kp, par = divmod(j, 2)
                    ps = pps.tile([128, 512], F32, name="gps")
                    for k in range(KT):
                        nc.tensor.matmul(
                            out=ps[:, 0:DC],
                            lhsT=hsT[:, k, j * 128:(j + 1) * 128],
                            rhs=wv[:, k, :],
                            start=(k == 0), stop=(k == KT - 1))
                    nc.vector.tensor_add(
                        out=vA[:, kp, par, :, 0:HD],
                        in0=ps[:, 0:DC].rearrange("p (h d) -> p h d", d=HD),
                        in1=bv_b.rearrange("p (h d) -> p h d", d=HD))

                def r_group(qt):
                    """Residual hs@Wp.T for query tile qt, natural layout,
                    straight into q-major x."""
                    ps = pps.tile([128, 512], F32, name="gps")
                    for k in range(KT):
                        nc.tensor.matmul(
                            out=ps[:, 0:DC],
                            lhsT=hsT[:, k, qt * 128:(qt + 1) * 128],
                            rhs=wp[:, k, :],
                            start=(k == 0), stop=(k == KT - 1))
                    nc.vector.tensor_copy(out=x[:, qt, :], in_=ps[:, 0:DC])

                def g_q(m, n):
                    return lambda: proj_group(wq, m, n, bq_s[:, m:m + 1], qT)

                def g_k(m, n):
                    return lambda: proj_group(wk, m, n, bk_s[:, m:m + 1], kT)

                def g_r(qt):
                    return lambda: r_group(qt)

                for g in [g_q(0, 0), g_k(0, 0)]:
                    g()
                blk0 = {1: g_k(0, 1), 3: g_k(0, 2), 5: g_k(0, 3),
                        7: g_q(0, 1), 9: g_r(0), 11: g_r(1), 13: g_r(2),
                        15: g_r(3)}
                fillers = [
                    g_q(0, 2), g_r(4), g_r(5), g_r(6), g_r(7), g_q(0, 3),
                    g_k(1, 0), g_k(1, 1),
                    g_k(1, 2), g_k(1, 3), g_q(1, 0), g_r(8), g_r(9),
                    g_r(10), g_r(11), g_q(1, 1),
                    g_q(1, 2), g_q(1, 3), g_r(12), g_r(13), g_r(14),
                    g_r(15),
                ]
                fillers.reverse()   # consumed via pop()

                cc_in = dram.tile([NS, 1024], F32)
                cc_out = dram.tile([NS, GROUPS * 1024], F32)
                cc_in23 = dram.tile([1, 2048], F32)   # qp-major: qp*16+(qn-2)*8+v*4+qt
                cc_out23 = dram.tile([1, GROUPS * 2048], F32)
                out_t = out_d.rearrange("(t p) c -> p t c", p=128)

                # ---------------- pipeline stages ----------------
                def s1a_xupdate(hp, qn, ctxc, den_r):
                    """x[q, head dims] += ctx * recip(denom); per-partition
                    scalars only. Deferred one block; pure DVE."""
                    for h2 in range(2):
                        g = 2 * hp + h2
                        for qt in range(4):
                            tmp = stg.tile([128, HD], F32, name="tmp")
                            nc.vector.tensor_scalar_mul(
                                out=tmp,
                                in0=ctxc[:, h2, qt * 128:qt * 128 + HD],
                                scalar1=den_r[:, h2, qt:qt + 1])
                            xs = x[:, 4 * qn + qt, g * HD:(g + 1) * HD]
                            nc.vector.tensor_tensor(
                                out=xs, in0=xs, in1=tmp, op=ALU.add)
                    if hp != MT - 1:
                        return None
                    xq = x[:, 4 * qn:4 * qn + 4, :]
                    xr = xrp.tile([128, 4, DC], F32, name="xr")
                    nc.vector.tensor_scalar_max(out=xr, in0=xq, scalar1=0.0)
                    x2 = xrp.tile([128, 4, DC], F32, name="x2")
                    nc.vector.tensor_tensor(
                        out=x2, in0=xr, in1=xr, op=ALU.mult)
                    return xr, x2

                def s1b_stats(qn, xr, x2, do_ag=True):
                    """local stats via free-dim reduces + AllGather issue.
                    Chunks 2 and 3 share one merged AllGather at the drain
                    (the two would otherwise serialize on the collective
                    device right at the end)."""
                    st = rows.tile([128, 2, 4], F32, name="st")
                    nc.vector.tensor_reduce(
                        out=st[:, 0, :], in_=xr, axis=mybir.AxisListType.X,
                        op=ALU.add)
                    nc.vector.tensor_reduce(
                        out=st[:, 1, :], in_=x2, axis=mybir.AxisListType.X,
                        op=ALU.add)
                    if not do_ag:
                        # qp-major staging for the merged chunk-2/3 gather
                        nc.sync.dma_start(
                            out=bass.AP(tensor=cc_in23.tensor,
                                        offset=cc_in23.offset + (qn - 2) * 8,
                                        ap=[[16, 128], [1, 8]]),
                            in_=st)
                        return
                    # dram layout per rank: [kind v][q = qt*128 + qp]
                    nc.sync.dma_start(
                        out=bass.AP(tensor=cc_in.tensor,
                                    offset=cc_in.offset + qn * 1024,
                                    ap=[[1, 128], [512, 2], [128, 4]]),
                        in_=st)
                    if do_ag:
                        nc.gpsimd.collective_compute(
                            "AllGather", ALU.bypass,
                            replica_groups=[[0, 1, 2, 3], [4, 5, 6, 7]],
                            ins=[cc_in[qn:qn + 1, :].opt()],
                            outs=[cc_out[qn:qn + 1, :].opt()],
                        )

                def s2_rowmath(qn, dmae=None, merged=False):
                    """group stats -> per-query A (rstd), B (mean*rstd)."""
                    dmae = dmae or nc.sync
                    if merged:
                        # rank stride 2048 != contiguous -> keep 3-dim APs
                        # on both sides (pad dest stride to 9 so it can't
                        # auto-merge)
                        rsb = rows.tile([128, GROUPS, 8], F32, name="rsb")
                        src_ap = bass.AP(
                            tensor=cc_out23.tensor,
                            offset=cc_out23.offset + (qn - 2) * 8,
                            ap=[[16, 128], [2048, GROUPS], [1, 8]])
                        dmae.dma_start(out=rsb, in_=src_ap)
                        rs = [rsb[:, r] for r in range(GROUPS)]
                    else:
                        rsb = rows.tile([128, GROUPS, 2, 4], F32, name="rsb")
                        src_ap = bass.AP(
                            tensor=cc_out.tensor,
                            offset=cc_out.offset + qn * GROUPS * 1024,
                            ap=[[1, 128], [1024, GROUPS], [512, 2],
                                [128, 4]])
                        dmae.dma_start(out=rsb, in_=src_ap)
                        rs = [rsb[:, r].rearrange("p a b -> p (a b)")
                              for r in range(GROUPS)]
                    acc = rows.tile([128, 8], F32, name="acc")
                    nc.vector.tensor_tensor(
                        out=acc, in0=rs[0], in1=rs[1], op=ALU.add)
                    nc.vector.tensor_tensor(
                        out=acc, in0=acc, in1=rs[2], op=ALU.add)
                    nc.vector.tensor_tensor(
                        out=acc, in0=acc, in1=rs[3], op=ALU.add)
                    mm = rows.tile([128, 8], F32, name="mm")
                    nc.vector.tensor_scalar_mul(
                        out=mm, in0=acc, scalar1=1.0 / D)
                    var = rows.tile([128, 4], F32, name="var")
                    nc.vector.tensor_tensor(
                        out=var, in0=mm[:, 0:4], in1=mm[:, 0:4],
                        op=ALU.mult)
                    nc.vector.tensor_tensor(
                        out=var, in0=mm[:, 4:8], in1=var, op=ALU.subtract)
                    sd = rows.tile([128, 4], F32, name="sd")
                    nc.scalar.activation(
                        out=sd, in_=var, func=AF.Sqrt, bias=eps_c)
                    AB = rows.tile([128, 2, 4], F32, name="AB")
                    nc.vector.reciprocal(out=AB[:, 0, :], in_=sd)
                    nc.vector.tensor_tensor(
                        out=AB[:, 1, :], in0=mm[:, 0:4], in1=AB[:, 0, :],
                        op=ALU.mult)
                    return AB

                def s3_apply(qn, AB, dmae=None, pool=False):
                    """out = (relu(x)*A - B)*gamma + beta, DMA out."""
                    dmae = dmae or nc.sync
                    for qt4 in range(4):
                        qt = 4 * qn + qt4
                        eng = nc.gpsimd if (pool and qt4 % 2) else nc.vector
                        y = stg.tile([128, DC], F32, name="y")
                        eng.tensor_scalar(
                            out=y, in0=x[:, qt, :],
                            scalar1=0.0, scalar2=AB[:, 0, qt4:qt4 + 1],
                            op0=ALU.max, op1=ALU.mult)
                        eng.tensor_scalar(
                            out=y, in0=y, scalar1=AB[:, 1, qt4:qt4 + 1],
                            scalar2=None, op0=ALU.subtract)
                        eng.tensor_tensor(
                            out=y, in0=y, in1=gm_b, op=ALU.mult)
                        eng.tensor_tensor(
                            out=x[:, qt, :], in0=y, in1=bt_b, op=ALU.add)
                    dmae.dma_start(
                        out=out_t[:, 4 * qn:4 * qn + 4, :],
                        in_=x[:, 4 * qn:4 * qn + 4, :])

                pend_xu = []      # (hp, qn, ctxc, den_r)
                pend_st = []      # (qn, xr, x2)
                pend_ag = []      # (qn, issue_block)
                pend_s3 = []      # (qn, AB)
                bi = 0

                def do_s1a():
                    if pend_xu:
                        hp_, qn_, ctxc_, den_ = pend_xu.pop(0)
                        r = s1a_xupdate(hp_, qn_, ctxc_, den_)
                        if r is not None:
                            pend_st.append((qn_, r[0], r[1]))

                def do_s1b(bi):
                    if pend_st:
                        qn_, xr_, x2_ = pend_st.pop(0)
                        s1b_stats(qn_, xr_, x2_, do_ag=True)
                        pend_ag.append((qn_, bi))

                def do_s2(bi, min_age=1, dmae=None):
                    if pend_ag and bi - pend_ag[0][1] >= min_age:
                        qn_, _ = pend_ag.pop(0)
                        pend_s3.append((qn_, s2_rowmath(qn_, dmae)))

                # ================= attention =================
                BLOCKS = [(0, 0), (0, 1), (0, 2), (0, 3),
                          (1, 0), (1, 1), (1, 2), (1, 3)]
                if True:
                    for hp, qn in BLOCKS:
                        qs = slice(qn * 512, (qn + 1) * 512)
                        # one PSUM bank hosts 4 accumulation groups (one
                        # per query tile): matmul start=True zeroing is
                        # bank-granular, so pre-zero via DVE and accumulate
                        # with start=False throughout.
                        ctxA = ctxps.tile([128, 512], F32, name="ctx")
                        ctxB = ctxps.tile([128, 512], F32, name="ctx")
                        nc.vector.memset(ctxA, 0.0)
                        nc.vector.memset(ctxB, 0.0)

                        def ctx_mms(pt, kp):
                            for h2, cps in ((0, ctxA), (1, ctxB)):
                                for qt4 in range(4):
                                    nc.tensor.matmul(
                                        out=cps[:, qt4 * 128:qt4 * 128 + HD + 1],
                                        lhsT=pt[:, :, h2,
                                                qt4 * 128:(qt4 + 1) * 128],
                                        rhs=vA[:, kp, :, 2 * hp + h2,
                                               0:HD + 1],
                                        start=False, stop=(kp == KP - 1),
                                        perf_mode=DR)

                        prev = None
                        for kp in range(KP):
                            pt = ptp.tile([128, 2, 2, 512], F8E5, name="pt")
                            for par in range(2):
                                ks = 2 * kp + par
                                sc = scps.tile([128, 1024], F32, name="sc")
                                kslc = slice(ks * 128, (ks + 1) * 128)
                                nc.tensor.matmul(
                                    out=sc[:, 0:512],
                                    lhsT=kT[0:64, hp, kslc],
                                    rhs=qT[0:64, hp, qs])
                                nc.tensor.matmul(
                                    out=sc[:, 512:1024],
                                    lhsT=kT[64:128, hp, kslc],
                                    rhs=qT[64:128, hp, qs])
                                nc.scalar.activation(
                                    out=pt[:, par], in_=sc, func=AF.Exp,
                                    scale=float(1.0 / np.sqrt(HD)))
                                if hp == 0 and qn == 0:
                                    v_group(ks)
                                    if ks in blk0:
                                        blk0[ks]()
                                elif fillers and ks % 2 == 0:
                                    fillers.pop()()
                            if prev is not None:
                                ctx_mms(*prev)
                                if kp == 1:
                                    do_s1a()
                                elif kp == 2:
                                    do_s1b(bi)
                                elif kp == 4:
                                    do_s2(bi)
                            prev = (pt, kp)
                        ctx_mms(*prev)

                        # denominators (per-partition!) + ctx copy-out
                        den_r = small.tile([128, 2, 4], F32, name="den")
                        for h2, cps in ((0, ctxA), (1, ctxB)):
                            nc.vector.reciprocal(
                                out=den_r[:, h2, :],
                                in_=bass.AP(tensor=cps.tensor,
                                            offset=cps.offset + HD,
                                            ap=[list(cps.ap[0]), [128, 4]]))
                        ctxc = stg.tile([128, 2, 512], F32, name="ctxc")
                        nc.vector.tensor_copy(out=ctxc[:, 0, :], in_=ctxA)
                        nc.vector.tensor_copy(out=ctxc[:, 1, :], in_=ctxB)
                        pend_xu.append((hp, qn, ctxc, den_r))
                        if pend_s3:
                            s3_apply(*pend_s3.pop(0))
                        bi += 1

                # drain
                do_s1a()
                do_s1b(bi)
                do_s2(bi, min_age=0)                   # qn=2
                do_s2(bi, min_age=0, dmae=nc.scalar)   # qn=3 (waits AG(3))
                s3_apply(*pend_s3.pop(0), pool=True)   # qn=2
                s3_apply(*pend_s3.pop(0), dmae=nc.scalar, pool=True)
            p1sb_cm.__exit__(None, None, None)
    _split_waits(nc)
    return nc


_NC = None
LAST_RESULT = None


def _get_nc():
    global _NC
    if _NC is None:
        _NC = build_bass()
    return _NC


def kernel(hidden_states, Wq, bq, Wk, bk, Wv, bv, Wp, gamma, beta):
    hs = np.asarray(hidden_states, dtype=np.float32)
    Wq = np.asarray(Wq, np.float32)
    Wk = np.asarray(Wk, np.float32)
    Wv = np.asarray(Wv, np.float32)
    Wp = np.asarray(Wp, np.float32)
    bq = np.asarray(bq, np.float32)
    bk = np.asarray(bk, np.float32)
    bv = np.asarray(bv, np.float32)
    gamma = np.asarray(gamma, np.float32)
    beta = np.asarray(beta, np.float32)
    bf = ml_dtypes.bfloat16

    nc = _get_nc()
    in_maps = []
    for c in range(NCORES):
        b, g = divmod(c, GROUPS)
        sl = slice(g * DC, (g + 1) * DC)
        in_maps.append({
            "hsT": np.ascontiguousarray(hs[b].T.astype(bf)),
            "wqT": np.ascontiguousarray(Wq[sl].T.astype(bf)),
            "wkT": np.ascontiguousarray(Wk[sl].T.astype(bf)),
            "wvT": np.ascontiguousarray(Wv[sl].T.astype(bf)),
            "wpT": np.ascontiguousarray(Wp[sl].T.astype(bf)),
            "bq": np.ascontiguousarray(bq[sl]),
            "bk": np.ascontiguousarray(bk[sl]),
            "bv": np.ascontiguousarray(bv[sl]),
            "gamma": np.ascontiguousarray(gamma[sl]),
            "beta": np.ascontiguousarray(beta[sl]),
        })
    res = run_bass_kernel_spmd(nc, in_maps, core_ids=list(range(NCORES)))
    global LAST_RESULT
    LAST_RESULT = res
    out = np.empty((B, S, D), np.float32)
    for c, r in enumerate(res.results):
        b, g = divmod(c, GROUPS)
        out[b, :, g * DC:(g + 1) * DC] = r["out"]
    return out


# revision 6
# speedup vs baseline: 1.0265x; 1.0129x over previous
"""Trainium2 Bass kernel for nn_BertAttention_78554951843978 (v3, q-major ctx).

Sharding: data-parallel over B (2 groups of 4 cores), tensor-parallel over
D within a group (256 dims = 4 heads per core).

Key structure:
  - hsT/weights in bf16; qT/kT bf16 (d-major, for scores)
  - probs fp8e5 from the exp activation; V fp8e4 in DoubleRow layout;
    ctx matmuls run fp8 DoubleRow TRANSPOSED: out[q, v-dim] with q on
    partitions, so the softmax denominator (ones column) and the
    layernorm A/B terms are all per-partition scalars -> no cross-
    partition broadcast bounces at all
  - x kept q-major [q % 128, qtile, D-slice]; residual projection done
    natural-layout (hs-stationary) straight into x
  - layernorm stats are DVE reduces (free-dim!) per query chunk,
    AllGather'ed per chunk (1.0x collective cost vs AllReduce's 1.875x),
    reduced locally, applied per chunk, pipelined behind the attention
  - PE warm-up spin at t=0 so the first real matmuls run at full clock
"""

import numpy as np
import ml_dtypes

import concourse.bass as bass
import concourse.tile as tile
from concourse import mybir
from concourse.bass_utils import run_bass_kernel_spmd

B, S, D, H = 2, 2048, 1024, 16
HD = 64
NCORES = 8
GROUPS = 4          # cores per batch
DC = D // GROUPS    # 256 dims per core
EPS = 1e-12
MP = 80             # V slot: 64 dims + 1 ones + pad to 16B slot stride

F32 = mybir.dt.float32
F32R = mybir.dt.float32r
BF16 = mybir.dt.bfloat16
F8E4 = mybir.dt.float8e4
F8E5 = mybir.dt.float8e5
AF = mybir.ActivationFunctionType
DR = mybir.MatmulPerfMode.DoubleRow
ALU = mybir.AluOpType

KT = D // 128    # 8 contraction tiles
MT = DC // 128   # 2 head pairs
NS = S // 512    # 4 query chunks of 512
ST = S // 128    # 16 key tiles
KP = ST // 2     # 8 key-tile pairs (DoubleRow)
QT = S // 128    # 16 query tiles of 128


def _split_waits(nc, keep=1):
    """Walrus rejects >1 sem wait per (non-EVSEM) instruction; hoist extras
    onto preceding single-wait NOPs on the same engine."""
    for bb in nc.main_func.blocks:
        insts = list(bb.instructions)
        out_list = []
        changed = False
        for inst in insts:
            si = inst.sync_info
            cap = 2 if isinstance(inst, mybir.InstEventSemaphore) else keep
            if si is not None and si.on_wait is not None and len(si.on_wait) > cap:
                waits = list(si.on_wait)
                for w in waits[cap:]:
                    out_list.append(mybir.InstNoOp(
                        name=nc.get_next_instruction_name(),
                        engine=inst.engine,
                        ins=[], outs=[],
                        sync_info=mybir.SyncInfo(on_wait=[w], on_update=[]),
                        bass_nofuse=True,
                    ))
                inst.sync_info = mybir.SyncInfo(
                    on_wait=waits[:cap], on_update=list(si.on_update or []))
                changed = True
            out_list.append(inst)
        if changed:
            bb.instructions = out_list


def build_bass():
    nc = bass.Bass(num_devices=NCORES)

    # ---------------- DRAM I/O ----------------
    hsT_d = nc.dram_tensor("hsT", [D, S], BF16, kind="ExternalInput")
    wqT_d = nc.dram_tensor("wqT", [D, DC], BF16, kind="ExternalInput")
    wkT_d = nc.dram_tensor("wkT", [D, DC], BF16, kind="ExternalInput")
    wvT_d = nc.dram_tensor("wvT", [D, DC], BF16, kind="ExternalInput")
    wpT_d = nc.dram_tensor("wpT", [D, DC], BF16, kind="ExternalInput")
    bq_d = nc.dram_tensor("bq", [DC], F32, kind="ExternalInput")
    bk_d = nc.dram_tensor("bk", [DC], F32, kind="ExternalInput")
    bv_d = nc.dram_tensor("bv", [DC], F32, kind="ExternalInput")
    gm_d = nc.dram_tensor("gamma", [DC], F32, kind="ExternalInput")
    bt_d = nc.dram_tensor("beta", [DC], F32, kind="ExternalInput")
    out_d = nc.dram_tensor("out", [S, DC], F32, kind="ExternalOutput")

    with tile.TileContext(nc) as tc:
        with (
            tc.tile_pool(name="persist", bufs=1) as persist,
            tc.tile_pool(name="dram", bufs=1, space="DRAM") as dram,
        ):
            # ------------- persistent SBUF -------------
            qT = persist.tile([128, MT, S], BF16)            # 8 KB/part
            kT = persist.tile([128, MT, S], BF16)
            x = persist.tile([128, QT, DC], F32)             # q-major, 16 KB
            # DoubleRow-packed aug V: [kp][parity][head g][MP]
            vA = persist.tile([128, KP, 2, GROUPS, MP], F8E4)
            # small constants: cols 0..3 = bq|bk (2 each), 4 = f32r ones,
            # 5 = eps, 6..261 = bv bcast, 262..517 gamma bcast, 518..773 beta
            cst = persist.tile([128, 6 + 3 * DC], F32)
            bq_s, bk_s = cst[:, 0:2], cst[:, 2:4]
            eps_c = cst[:, 5:6]
            bv_b = cst[:, 6:6 + DC]
            gm_b = cst[:, 6 + DC:6 + 2 * DC]
            bt_b = cst[:, 6 + 2 * DC:6 + 3 * DC]
            wsrc = persist.tile([128, 256], BF16)            # warm-up fodder
            onesr = persist.tile([128, 1], BF16)

            p1sb_cm = tc.tile_pool(name="p1sb", bufs=1)
            p1sb = p1sb_cm.__enter__()
            hsT = p1sb.tile([128, KT, S], BF16)              # 32 KB/part
            wq = p1sb.tile([128, KT, MT, 128], BF16)
            wk = p1sb.tile([128, KT, MT, 128], BF16)
            wv = p1sb.tile([128, KT, DC], BF16)
            wp = p1sb.tile([128, KT, DC], BF16)

            # input DMAs: few, large transfers (desc-gen is serialized);
            # first-needed slices first; scalar queue untouched.
            hsT_t = hsT_d.rearrange("(t p) s -> p t s", p=128)
            wq_t = wqT_d.rearrange("(t p) (m f) -> p t m f", p=128, f=128)
            wk_t = wkT_d.rearrange("(t p) (m f) -> p t m f", p=128, f=128)
            wv_t = wvT_d.rearrange("(t p) c -> p t c", p=128)
            wp_t = wpT_d.rearrange("(t p) c -> p t c", p=128)
            nc.gpsimd.dma_start(out=bq_s, in_=bq_d.rearrange("(m p) -> p m", p=128))
            nc.gpsimd.dma_start(out=bk_s, in_=bk_d.rearrange("(m p) -> p m", p=128))
            nc.sync.dma_start(out=wq, in_=wq_t)
            for k in range(KT):
                e = nc.sync if k % 2 == 0 else nc.gpsimd
                e.dma_start(out=hsT[:, k, 0:512], in_=hsT_t[:, k, 0:512])
            nc.gpsimd.dma_start(out=wk, in_=wk_t)
            nc.gpsimd.dma_start(out=bv_b, in_=bass.AP(
                tensor=bv_d[:].tensor, offset=0, ap=[[0, 128], [1, DC]]))
            nc.sync.dma_start(out=wv, in_=wv_t)
            for k in range(KT):
                e = nc.sync if k % 2 == 0 else nc.gpsimd
                e.dma_start(out=hsT[:, k, 512:1024], in_=hsT_t[:, k, 512:1024])
            for k in range(KT):
                e = nc.sync if k % 2 == 0 else nc.gpsimd
                e.dma_start(out=hsT[:, k, 1024:2048],
                            in_=hsT_t[:, k, 1024:2048])
            nc.gpsimd.dma_start(out=wp, in_=wp_t)
            nc.gpsimd.dma_start(out=gm_b, in_=bass.AP(
                tensor=gm_d[:].tensor, offset=0, ap=[[0, 128], [1, DC]]))
            nc.gpsimd.dma_start(out=bt_b, in_=bass.AP(
                tensor=bt_d[:].tensor, offset=0, ap=[[0, 128], [1, DC]]))
            nc.vector.memset(onesr, 1.0)
            nc.vector.memset(eps_c, EPS)
            nc.vector.memset(wsrc, 1.0)
            nc.vector.memset(vA[:, :, :, :, HD:MP], 0.0)
            nc.vector.memset(vA[:, :, :, :, HD:HD + 1], 1.0)

            with (
                tc.tile_pool(name="pps", bufs=2, space="PSUM") as pps,
                tc.tile_pool(name="scps", bufs=2, space="PSUM") as scps,
                tc.tile_pool(name="ctxps", bufs=2, space="PSUM") as ctxps,
                tc.tile_pool(name="ptp", bufs=4) as ptp,
                tc.tile_pool(name="small", bufs=2) as small,
                tc.tile_pool(name="stg", bufs=2) as stg,
                tc.tile_pool(name="xrp", bufs=2) as xrp,
                tc.tile_pool(name="rows", bufs=2) as rows,
            ):
                # PE warm-up: ~28 cheap matmuls so pe ramps to full clock
                # before the first real projection (which waits on DMAs).
                wps = pps.tile([128, 512], F32, name="gps")
                for i in range(24):
                    nc.tensor.matmul(out=wps[0:1, 0:256], lhsT=onesr,
                                     rhs=wsrc, start=True, stop=True)

                def proj_group(w_sb, m, n, bias, out_sb):
                    """One [128,512] block of a W-stationary projection."""
                    ps = pps.tile([128, 512], F32, name="gps")
                    for k in range(KT):
                        nc.tensor.matmul(
                            out=ps, lhsT=w_sb[:, k, m, :],
                            rhs=hsT[:, k, n * 512:(n + 1) * 512],
                            start=(k == 0), stop=(k == KT - 1))
                    o = out_sb[:, m, n * 512:(n + 1) * 512]
                    nc.vector.tensor_scalar_add(out=o, in0=ps, scalar1=bias)

                def v_group(j):
                    """V for key tile j -> fp8e4 DoubleRow slot, + bias."""
                    kp, par = divmod(j, 2)
                    ps = pps.tile([128, 512], F32, name="gps")
                    for k in range(KT):
                        nc.tensor.matmul(
                            out=ps[:, 0:DC],
                            lhsT=hsT[:, k, j * 128:(j + 1) * 128],
                            rhs=wv[:, k, :],
                            start=(k == 0), stop=(k == KT - 1))
                    nc.vector.tensor_add(
                        out=vA[:, kp, par, :, 0:HD],
                        in0=ps[:, 0:DC].rearrange("p (h d) -> p h d", d=HD),
                        in1=bv_b.rearrange("p (h d) -> p h d", d=HD))

                def r_group(qt):
                    """Residual hs@Wp.T for query tile qt, natural layout,
                    straight into q-major x."""
                    ps = pps.tile([128, 512], F32, name="gps")
                    for k in range(KT):
                        nc.tensor.matmul(
                            out=ps[:, 0:DC],
                            lhsT=hsT[:, k, qt * 128:(qt + 1) * 128],
                            rhs=wp[:, k, :],
                            start=(k == 0), stop=(k == KT - 1))
                    nc.vector.tensor_copy(out=x[:, qt, :], in_=ps[:, 0:DC])

                def g_q(m, n):
                    return lambda: proj_group(wq, m, n, bq_s[:, m:m + 1], qT)

                def g_k(m, n):
                    return lambda: proj_group(wk, m, n, bk_s[:, m:m + 1], kT)

                def g_r(qt):
                    return lambda: r_group(qt)

                for g in [g_q(0, 0), g_k(0, 0)]:
                    g()
                blk0 = {1: g_k(0, 1), 3: g_k(0, 2), 5: g_k(0, 3),
                        7: g_q(0, 1), 9: g_r(0), 11: g_r(1), 13: g_r(2),
                        15: g_r(3)}
                fillers = [
                    g_q(0, 2), g_r(4), g_r(5), g_r(6), g_r(7), g_q(0, 3),
                    g_k(1, 0), g_k(1, 1),
                    g_k(1, 2), g_k(1, 3), g_q(1, 0), g_r(8), g_r(9),
                    g_r(10), g_r(11), g_q(1, 1),
                    g_q(1, 2), g_q(1, 3), g_r(12), g_r(13), g_r(14),
                    g_r(15),
                ]
                fillers.reverse()   # consumed via pop()

                cc_in = dram.tile([NS, 1024], F32)
                cc_out = dram.tile([NS, GROUPS * 1024], F32)
                cc_in23 = dram.tile([1, 2048], F32)   # qp-major: qp*16+(qn-2)*8+v*4+qt
                cc_out23 = dram.tile([1, GROUPS * 2048], F32)
                out_t = out_d.rearrange("(t p) c -> p t c", p=128)

                # ---------------- pipeline stages ----------------
                def s1a_xupdate(hp, qn, ctxc, den_r):
                    """x[q, head dims] += ctx * recip(denom); per-partition
                    scalars only. Deferred one block; pure DVE."""
                    for h2 in range(2):
                        g = 2 * hp + h2
                        for qt in range(4):
                            tmp = stg.tile([128, HD], F32, name="tmp")
                            nc.vector.tensor_scalar_mul(
                                out=tmp,
                                in0=ctxc[:, h2, qt * 128:qt * 128 + HD],
                                scalar1=den_r[:, h2, qt:qt + 1])
                            xs = x[:, 4 * qn + qt, g * HD:(g + 1) * HD]
                            nc.vector.tensor_tensor(
                                out=xs, in0=xs, in1=tmp, op=ALU.add)
                    if hp != MT - 1:
                        return None
                    xq = x[:, 4 * qn:4 * qn + 4, :]
                    xr = xrp.tile([128, 4, DC], F32, name="xr")
                    nc.vector.tensor_scalar_max(out=xr, in0=xq, scalar1=0.0)
                    x2 = xrp.tile([128, 4, DC], F32, name="x2")
                    nc.vector.tensor_tensor(
                        out=x2, in0=xr, in1=xr, op=ALU.mult)
                    return xr, x2

                def s1b_stats(qn, xr, x2, do_ag=True):
                    """local stats via free-dim reduces + AllGather issue.
                    Chunks 2 and 3 share one merged AllGather at the drain
                    (the two would otherwise serialize on the collective
                    device right at the end)."""
                    st = rows.tile([128, 2, 4], F32, name="st")
                    nc.vector.tensor_reduce(
                        out=st[:, 0, :], in_=xr, axis=mybir.AxisListType.X,
                        op=ALU.add)
                    nc.vector.tensor_reduce(
                        out=st[:, 1, :], in_=x2, axis=mybir.AxisListType.X,
                        op=ALU.add)
                    if not do_ag:
                        # qp-major staging for the merged chunk-2/3 gather
                        nc.sync.dma_start(
                            out=bass.AP(tensor=cc_in23.tensor,
                                        offset=cc_in23.offset + (qn - 2) * 8,
                                        ap=[[16, 128], [1, 8]]),
                            in_=st)
                        return
                    # dram layout per rank: [kind v][q = qt*128 + qp]
                    nc.sync.dma_start(
                        out=bass.AP(tensor=cc_in.tensor,
                                    offset=cc_in.offset + qn * 1024,
                                    ap=[[1, 128], [512, 2], [128, 4]]),
                        in_=st)
                    if do_ag:
                        nc.gpsimd.collective_compute(
                            "AllGather", ALU.bypass,
                            replica_groups=[[0, 1, 2, 3], [4, 5, 6, 7]],
                            ins=[cc_in[qn:qn + 1, :].opt()],
                            outs=[cc_out[qn:qn + 1, :].opt()],
                        )

                def s2_rowmath(qn, dmae=None, merged=False):
                    """group stats -> per-query A (rstd), B (mean*rstd)."""
                    dmae = dmae or nc.sync
                    if merged:
                        # rank stride 2048 != contiguous -> keep 3-dim APs
                        # on both sides (pad dest stride to 9 so it can't
                        # auto-merge)
                        rsb = rows.tile([128, GROUPS, 8], F32, name="rsb")
                        src_ap = bass.AP(
                            tensor=cc_out23.tensor,
                            offset=cc_out23.offset + (qn - 2) * 8,
                            ap=[[16, 128], [2048, GROUPS], [1, 8]])
                        dmae.dma_start(out=rsb, in_=src_ap)
                        rs = [rsb[:, r] for r in range(GROUPS)]
                    else:
                        rsb = rows.tile([128, GROUPS, 2, 4], F32, name="rsb")
                        src_ap = bass.AP(
                            tensor=cc_out.tensor,
                            offset=cc_out.offset + qn * GROUPS * 1024,
                            ap=[[1, 128], [1024, GROUPS], [512, 2],
                                [128, 4]])
                        dmae.dma_start(out=rsb, in_=src_ap)
                        rs = [rsb[:, r].rearrange("p a b -> p (a b)")
                              for r in range(GROUPS)]
                    acc = rows.tile([128, 8], F32, name="acc")
                    nc.vector.tensor_tensor(
                        out=acc, in0=rs[0], in1=rs[1], op=ALU.add)
                    nc.vector.tensor_tensor(
                        out=acc, in0=acc, in1=rs[2], op=ALU.add)
                    nc.vector.tensor_tensor(
                        out=acc, in0=acc, in1=rs[3], op=ALU.add)
                    mm = rows.tile([128, 8], F32, name="mm")
                    nc.vector.tensor_scalar_mul(
                        out=mm, in0=acc, scalar1=1.0 / D)
                    var = rows.tile([128, 4], F32, name="var")
                    nc.vector.tensor_tensor(
                        out=var, in0=mm[:, 0:4], in1=mm[:, 0:4],
                        op=ALU.mult)
                    nc.vector.tensor_tensor(
                        out=var, in0=mm[:, 4:8], in1=var, op=ALU.subtract)
                    sd = rows.tile([128, 4], F32, name="sd")
                    nc.scalar.activation(
                        out=sd, in_=var, func=AF.Sqrt, bias=eps_c)
                    AB = rows.tile([128, 2, 4], F32, name="AB")
                    nc.vector.reciprocal(out=AB[:, 0, :], in_=sd)
                    nc.vector.tensor_tensor(
                        out=AB[:, 1, :], in0=mm[:, 0:4], in1=AB[:, 0, :],
                        op=ALU.mult)
                    return AB

                def s3_apply(qn, AB, dmae=None, pool=False):
                    """out = (relu(x)*A - B)*gamma + beta, DMA out."""
                    dmae = dmae or nc.sync
                    for qt4 in range(4):
                        qt = 4 * qn + qt4
                        eng = nc.gpsimd if (pool and qt4 % 2) else nc.vector
                        y = stg.tile([128, DC], F32, name="y")
                        eng.tensor_scalar(
                            out=y, in0=x[:, qt, :],
                            scalar1=0.0, scalar2=AB[:, 0, qt4:qt4 + 1],
                            op0=ALU.max, op1=ALU.mult)
                        eng.tensor_scalar(
                            out=y, in0=y, scalar1=AB[:, 1, qt4:qt4 + 1],
                            scalar2=None, op0=ALU.subtract)
                        eng.tensor_tensor(
                            out=y, in0=y, in1=gm_b, op=ALU.mult)
                        eng.tensor_tensor(
                            out=x[:, qt, :], in0=y, in1=bt_b, op=ALU.add)
                        if qt4 == 1:
                            dmae.dma_start(
                                out=out_t[:, 4 * qn:4 * qn + 2, :],
                                in_=x[:, 4 * qn:4 * qn + 2, :])
                    dmae.dma_start(
                        out=out_t[:, 4 * qn + 2:4 * qn + 4, :],
                        in_=x[:, 4 * qn + 2:4 * qn + 4, :])

                pend_xu = []      # (hp, qn, ctxc, den_r)
                pend_st = []      # (qn, xr, x2)
                pend_ag = []      # (qn, issue_block)
                pend_s3 = []      # (qn, AB)
                bi = 0

                def do_s1a():
                    if pend_xu:
                        hp_, qn_, ctxc_, den_ = pend_xu.pop(0)
                        r = s1a_xupdate(hp_, qn_, ctxc_, den_)
                        if r is not None:
                            pend_st.append((qn_, r[0], r[1]))

                def do_s1b(bi):
                    if pend_st:
                        qn_, xr_, x2_ = pend_st.pop(0)
                        s1b_stats(qn_, xr_, x2_, do_ag=True)
                        pend_ag.append((qn_, bi))

                def do_s2(bi, min_age=1, dmae=None):
                    if pend_ag and bi - pend_ag[0][1] >= min_age:
                        qn_, _ = pend_ag.pop(0)
                        pend_s3.append((qn_, s2_rowmath(qn_, dmae)))

                # ================= attention =================
                BLOCKS = [(0, 0), (0, 1), (0, 2), (0, 3),
                          (1, 0), (1, 1), (1, 2), (1, 3)]
                if True:
                    for hp, qn in BLOCKS:
                        qs = slice(qn * 512, (qn + 1) * 512)
                        # one PSUM bank hosts 4 accumulation groups (one
                        # per query tile): matmul start=True zeroing is
                        # bank-granular, so pre-zero via DVE and accumulate
                        # with start=False throughout.
                        ctxA = ctxps.tile([128, 512], F32, name="ctx")
                        ctxB = ctxps.tile([128, 512], F32, name="ctx")
                        nc.vector.memset(ctxA, 0.0)
                        nc.vector.memset(ctxB, 0.0)

                        def ctx_mms(pt, kp):
                            for h2, cps in ((0, ctxA), (1, ctxB)):
                                for qt4 in range(4):
                                    nc.tensor.matmul(
                                        out=cps[:, qt4 * 128:qt4 * 128 + HD + 1],
                                        lhsT=pt[:, :, h2,
                                                qt4 * 128:(qt4 + 1) * 128],
                                        rhs=vA[:, kp, :, 2 * hp + h2,
                                               0:HD + 1],
                                        start=False, stop=(kp == KP - 1),
                                        perf_mode=DR)

                        prev = None
                        for kp in range(KP):
                            pt = ptp.tile([128, 2, 2, 512], F8E5, name="pt")
                            for par in range(2):
                                ks = 2 * kp + par
                                sc = scps.tile([128, 1024], F32, name="sc")
                                kslc = slice(ks * 128, (ks + 1) * 128)
                                nc.tensor.matmul(
                                    out=sc[:, 0:512],
                                    lhsT=kT[0:64, hp, kslc],
                                    rhs=qT[0:64, hp, qs])
                                nc.tensor.matmul(
                                    out=sc[:, 512:1024],
                                    lhsT=kT[64:128, hp, kslc],
                                    rhs=qT[64:128, hp, qs])
                                nc.scalar.activation(
                                    out=pt[:, par], in_=sc, func=AF.Exp,
                                    scale=float(1.0 / np.sqrt(HD)))
                                if hp == 0 and qn == 0:
                                    v_group(ks)
                                    if ks in blk0:
                                        blk0[ks]()
                                elif fillers and ks % 2 == 0:
                                    fillers.pop()()
                            if prev is not None:
                                ctx_mms(*prev)
                                if kp == 1:
                                    do_s1a()
                                elif kp == 2:
                                    do_s1b(bi)
                                elif kp == 4:
                                    do_s2(bi)
                            prev = (pt, kp)
                        ctx_mms(*prev)

                        # denominators (per-partition!) + ctx copy-out
                        den_r = small.tile([128, 2, 4], F32, name="den")
                        for h2, cps in ((0, ctxA), (1, ctxB)):
                            nc.vector.reciprocal(
                                out=den_r[:, h2, :],
                                in_=bass.AP(tensor=cps.tensor,
                                            offset=cps.offset + HD,
                                            ap=[list(cps.ap[0]), [128, 4]]))
                        ctxc = stg.tile([128, 2, 512], F32, name="ctxc")
                        nc.vector.tensor_copy(out=ctxc[:, 0, :], in_=ctxA)
                        nc.vector.tensor_copy(out=ctxc[:, 1, :], in_=ctxB)
                        pend_xu.append((hp, qn, ctxc, den_r))
                        if pend_s3:
                            s3_apply(*pend_s3.pop(0))
                        bi += 1

                # drain
                do_s1a()
                do_s1b(bi)
                do_s2(bi, min_age=0)                   # qn=2
                do_s2(bi, min_age=0, dmae=nc.scalar)   # qn=3 (waits AG(3))
                s3_apply(*pend_s3.pop(0), pool=True)   # qn=2
                s3_apply(*pend_s3.pop(0), dmae=nc.scalar, pool=True)
            p1sb_cm.__exit__(None, None, None)
    _split_waits(nc)
    return nc


_NC = None
LAST_RESULT = None


def _get_nc():
    global _NC
    if _NC is None:
        _NC = build_bass()
    return _NC


def kernel(hidden_states, Wq, bq, Wk, bk, Wv, bv, Wp, gamma, beta):
    hs = np.asarray(hidden_states, dtype=np.float32)
    Wq = np.asarray(Wq, np.float32)
    Wk = np.asarray(Wk, np.float32)
    Wv = np.asarray(Wv, np.float32)
    Wp = np.asarray(Wp, np.float32)
    bq = np.asarray(bq, np.float32)
    bk = np.asarray(bk, np.float32)
    bv = np.asarray(bv, np.float32)
    gamma = np.asarray(gamma, np.float32)
    beta = np.asarray(beta, np.float32)
    bf = ml_dtypes.bfloat16

    nc = _get_nc()
    in_maps = []
    for c in range(NCORES):
        b, g = divmod(c, GROUPS)
        sl = slice(g * DC, (g + 1) * DC)
        in_maps.append({
            "hsT": np.ascontiguousarray(hs[b].T.astype(bf)),
            "wqT": np.ascontiguousarray(Wq[sl].T.astype(bf)),
            "wkT": np.ascontiguousarray(Wk[sl].T.astype(bf)),
            "wvT": np.ascontiguousarray(Wv[sl].T.astype(bf)),
            "wpT": np.ascontiguousarray(Wp[sl].T.astype(bf)),
            "bq": np.ascontiguousarray(bq[sl]),
            "bk": np.ascontiguousarray(bk[sl]),
            "bv": np.ascontiguousarray(bv[sl]),
            "gamma": np.ascontiguousarray(gamma[sl]),
            "beta": np.ascontiguousarray(beta[sl]),
        })
    res = run_bass_kernel_spmd(nc, in_maps, core_ids=list(range(NCORES)))
    global LAST_RESULT
    LAST_RESULT = res
    out = np.empty((B, S, D), np.float32)
    for c, r in enumerate(res.results):
        b, g = divmod(c, GROUPS)
        out[b, :, g * DC:(g + 1) * DC] = r["out"]
    return out


# revision 7
# speedup vs baseline: 1.0572x; 1.0299x over previous
"""Trainium2 Bass kernel for nn_BertAttention_78554951843978 (v3, q-major ctx).

Sharding: data-parallel over B (2 groups of 4 cores), tensor-parallel over
D within a group (256 dims = 4 heads per core).

Key structure:
  - hsT/weights in bf16; qT/kT bf16 (d-major, for scores)
  - probs fp8e5 from the exp activation; V fp8e4 in DoubleRow layout;
    ctx matmuls run fp8 DoubleRow TRANSPOSED: out[q, v-dim] with q on
    partitions, so the softmax denominator (ones column) and the
    layernorm A/B terms are all per-partition scalars -> no cross-
    partition broadcast bounces at all
  - x kept q-major [q % 128, qtile, D-slice]; residual projection done
    natural-layout (hs-stationary) straight into x
  - layernorm stats are DVE reduces (free-dim!) per query chunk,
    AllGather'ed per chunk (1.0x collective cost vs AllReduce's 1.875x),
    reduced locally, applied per chunk, pipelined behind the attention
  - PE warm-up spin at t=0 so the first real matmuls run at full clock
"""

import numpy as np
import ml_dtypes

import concourse.bass as bass
import concourse.tile as tile
from concourse import mybir
from concourse.bass_utils import run_bass_kernel_spmd

B, S, D, H = 2, 2048, 1024, 16
HD = 64
NCORES = 8
GROUPS = 4          # cores per batch
DC = D // GROUPS    # 256 dims per core
EPS = 1e-12
MP = 80             # V slot: 64 dims + 1 ones + pad to 16B slot stride

F32 = mybir.dt.float32
F32R = mybir.dt.float32r
BF16 = mybir.dt.bfloat16
F8E4 = mybir.dt.float8e4
F8E5 = mybir.dt.float8e5
AF = mybir.ActivationFunctionType
DR = mybir.MatmulPerfMode.DoubleRow
ALU = mybir.AluOpType

KT = D // 128    # 8 contraction tiles
MT = DC // 128   # 2 head pairs
NS = S // 512    # 4 query chunks of 512
ST = S // 128    # 16 key tiles
KP = ST // 2     # 8 key-tile pairs (DoubleRow)
QT = S // 128    # 16 query tiles of 128


def _split_waits(nc, keep=1):
    """Walrus rejects >1 sem wait per (non-EVSEM) instruction; hoist extras
    onto preceding single-wait NOPs on the same engine."""
    for bb in nc.main_func.blocks:
        insts = list(bb.instructions)
        out_list = []
        changed = False
        for inst in insts:
            si = inst.sync_info
            cap = 2 if isinstance(inst, mybir.InstEventSemaphore) else keep
            if si is not None and si.on_wait is not None and len(si.on_wait) > cap:
                waits = list(si.on_wait)
                for w in waits[cap:]:
                    out_list.append(mybir.InstNoOp(
                        name=nc.get_next_instruction_name(),
                        engine=inst.engine,
                        ins=[], outs=[],
                        sync_info=mybir.SyncInfo(on_wait=[w], on_update=[]),
                        bass_nofuse=True,
                    ))
                inst.sync_info = mybir.SyncInfo(
                    on_wait=waits[:cap], on_update=list(si.on_update or []))
                changed = True
            out_list.append(inst)
        if changed:
            bb.instructions = out_list


def build_bass():
    nc = bass.Bass(num_devices=NCORES)

    # ---------------- DRAM I/O ----------------
    hsT_d = nc.dram_tensor("hsT", [D, S], BF16, kind="ExternalInput")
    hsT8_d = nc.dram_tensor("hsT8", [D, S], F8E4, kind="ExternalInput")
    wvT8_d = nc.dram_tensor("wvT8", [D, DC], F8E4, kind="ExternalInput")
    wqT_d = nc.dram_tensor("wqT", [D, DC], BF16, kind="ExternalInput")
    wkT_d = nc.dram_tensor("wkT", [D, DC], BF16, kind="ExternalInput")
    wpT_d = nc.dram_tensor("wpT", [D, DC], BF16, kind="ExternalInput")
    bq_d = nc.dram_tensor("bq", [DC], F32, kind="ExternalInput")
    bk_d = nc.dram_tensor("bk", [DC], F32, kind="ExternalInput")
    bv_d = nc.dram_tensor("bv", [DC], F32, kind="ExternalInput")
    gm_d = nc.dram_tensor("gamma", [DC], F32, kind="ExternalInput")
    bt_d = nc.dram_tensor("beta", [DC], F32, kind="ExternalInput")
    out_d = nc.dram_tensor("out", [S, DC], F32, kind="ExternalOutput")

    with tile.TileContext(nc) as tc:
        with (
            tc.tile_pool(name="persist", bufs=1) as persist,
            tc.tile_pool(name="dram", bufs=1, space="DRAM") as dram,
        ):
            # ------------- persistent SBUF -------------
            qT = persist.tile([128, MT, S], BF16)            # 8 KB/part
            kT = persist.tile([128, MT, S], BF16)
            x = persist.tile([128, QT, DC], F32)             # q-major, 16 KB
            # DoubleRow-packed aug V: [kp][parity][head g][MP]
            vA = persist.tile([128, KP, 2, GROUPS, MP], F8E4)
            # small constants: cols 0..3 = bq|bk (2 each), 4 = f32r ones,
            # 5 = eps, 6..261 = bv bcast, 262..517 gamma bcast, 518..773 beta
            cst = persist.tile([128, 6 + 3 * DC], F32)
            bq_s, bk_s = cst[:, 0:2], cst[:, 2:4]
            eps_c = cst[:, 5:6]
            bv_b = cst[:, 6:6 + DC]
            gm_b = cst[:, 6 + DC:6 + 2 * DC]
            bt_b = cst[:, 6 + 2 * DC:6 + 3 * DC]
            wsrc = persist.tile([128, 256], BF16)            # warm-up fodder
            onesr = persist.tile([128, 1], BF16)

            p1sb_cm = tc.tile_pool(name="p1sb", bufs=1)
            p1sb = p1sb_cm.__enter__()
            hsT = p1sb.tile([128, KT, S], BF16)              # 32 KB/part
            hsT8 = p1sb.tile([128, KT, S], F8E4)             # 16 KB/part
            wv8 = p1sb.tile([128, KT, DC], F8E4)
            # (bf16 wv no longer needed: V runs on the fp8 path)
            wq = p1sb.tile([128, KT, MT, 128], BF16)
            wk = p1sb.tile([128, KT, MT, 128], BF16)
            wp = p1sb.tile([128, KT, DC], BF16)

            # input DMAs: few, large transfers (desc-gen is serialized);
            # first-needed slices first; scalar queue untouched.
            hsT_t = hsT_d.rearrange("(t p) s -> p t s", p=128)
            wq_t = wqT_d.rearrange("(t p) (m f) -> p t m f", p=128, f=128)
            wk_t = wkT_d.rearrange("(t p) (m f) -> p t m f", p=128, f=128)
            wp_t = wpT_d.rearrange("(t p) c -> p t c", p=128)
            nc.gpsimd.dma_start(out=bq_s, in_=bq_d.rearrange("(m p) -> p m", p=128))
            nc.gpsimd.dma_start(out=bk_s, in_=bk_d.rearrange("(m p) -> p m", p=128))
            nc.sync.dma_start(out=wq, in_=wq_t)
            for k in range(KT):
                e = nc.sync if k % 2 == 0 else nc.gpsimd
                e.dma_start(out=hsT[:, k, 0:512], in_=hsT_t[:, k, 0:512])
            nc.gpsimd.dma_start(out=wk, in_=wk_t)
            nc.gpsimd.dma_start(out=bv_b, in_=bass.AP(
                tensor=bv_d[:].tensor, offset=0, ap=[[0, 128], [1, DC]]))
            nc.sync.dma_start(
                out=wv8, in_=wvT8_d.rearrange("(t p) c -> p t c", p=128))
            hsT8_t = hsT8_d.rearrange("(t p) s -> p t s", p=128)
            for k in range(KT):
                e = nc.sync if k % 2 == 0 else nc.gpsimd
                e.dma_start(out=hsT8[:, k, 0:512], in_=hsT8_t[:, k, 0:512])
                e.dma_start(out=hsT[:, k, 512:1024], in_=hsT_t[:, k, 512:1024])
            for k in range(KT):
                e = nc.sync if k % 2 == 0 else nc.gpsimd
                e.dma_start(out=hsT8[:, k, 512:2048], in_=hsT8_t[:, k, 512:2048])
            for k in range(KT):
                e = nc.sync if k % 2 == 0 else nc.gpsimd
                e.dma_start(out=hsT[:, k, 1024:2048],
                            in_=hsT_t[:, k, 1024:2048])
            nc.gpsimd.dma_start(out=wp, in_=wp_t)
            nc.gpsimd.dma_start(out=gm_b, in_=bass.AP(
                tensor=gm_d[:].tensor, offset=0, ap=[[0, 128], [1, DC]]))
            nc.gpsimd.dma_start(out=bt_b, in_=bass.AP(
                tensor=bt_d[:].tensor, offset=0, ap=[[0, 128], [1, DC]]))
            nc.vector.memset(onesr, 1.0)
            nc.vector.memset(eps_c, EPS)
            nc.vector.memset(wsrc, 1.0)
            nc.vector.memset(vA[:, :, :, :, HD:MP], 0.0)
            nc.vector.memset(vA[:, :, :, :, HD:HD + 1], 1.0)

            with (
                tc.tile_pool(name="pps", bufs=2, space="PSUM") as pps,
                tc.tile_pool(name="scps", bufs=2, space="PSUM") as scps,
                tc.tile_pool(name="ctxps", bufs=2, space="PSUM") as ctxps,
                tc.tile_pool(name="ptp", bufs=4) as ptp,
                tc.tile_pool(name="small", bufs=2) as small,
                tc.tile_pool(name="stg", bufs=2) as stg,
                tc.tile_pool(name="xrp", bufs=2) as xrp,
                tc.tile_pool(name="rows", bufs=2) as rows,
            ):
                # PE warm-up: ~28 cheap matmuls so pe ramps to full clock
                # before the first real projection (which waits on DMAs).
                wps = pps.tile([128, 512], F32, name="gps")
                for i in range(24):
                    nc.tensor.matmul(out=wps[0:1, 0:256], lhsT=onesr,
                                     rhs=wsrc, start=True, stop=True)

                def proj_group(w_sb, m, n, bias, out_sb):
                    """One [128,512] block of a W-stationary projection."""
                    ps = pps.tile([128, 512], F32, name="gps")
                    for k in range(KT):
                        nc.tensor.matmul(
                            out=ps, lhsT=w_sb[:, k, m, :],
                            rhs=hsT[:, k, n * 512:(n + 1) * 512],
                            start=(k == 0), stop=(k == KT - 1))
                    o = out_sb[:, m, n * 512:(n + 1) * 512]
                    nc.vector.tensor_scalar_add(out=o, in0=ps, scalar1=bias)

                def v_group(j):
                    """V for key tile j via fp8 DoubleRow (4x fewer PE
                    cycles; V is quantized to fp8e4 downstream anyway)."""
                    kp, par = divmod(j, 2)
                    ps = pps.tile([128, 512], F32, name="gps")
                    for t in range(KT // 2):
                        nc.tensor.matmul(
                            out=ps[:, 0:DC],
                            lhsT=hsT8[:, 2 * t:2 * t + 2,
                                      j * 128:(j + 1) * 128],
                            rhs=wv8[:, 2 * t:2 * t + 2, :],
                            start=(t == 0), stop=(t == KT // 2 - 1),
                            perf_mode=DR)
                    nc.vector.tensor_add(
                        out=vA[:, kp, par, :, 0:HD],
                        in0=ps[:, 0:DC].rearrange("p (h d) -> p h d", d=HD),
                        in1=bv_b.rearrange("p (h d) -> p h d", d=HD))

                def r_group(qt):
                    """Residual hs@Wp.T for query tile qt, natural layout,
                    straight into q-major x."""
                    ps = pps.tile([128, 512], F32, name="gps")
                    for k in range(KT):
                        nc.tensor.matmul(
                            out=ps[:, 0:DC],
                            lhsT=hsT[:, k, qt * 128:(qt + 1) * 128],
                            rhs=wp[:, k, :],
                            start=(k == 0), stop=(k == KT - 1))
                    nc.vector.tensor_copy(out=x[:, qt, :], in_=ps[:, 0:DC])

                def g_q(m, n):
                    return lambda: proj_group(wq, m, n, bq_s[:, m:m + 1], qT)

                def g_k(m, n):
                    return lambda: proj_group(wk, m, n, bk_s[:, m:m + 1], kT)

                def g_r(qt):
                    return lambda: r_group(qt)

                for g in [g_q(0, 0), g_k(0, 0)]:
                    g()
                blk0 = {1: g_k(0, 1), 3: g_k(0, 2), 5: g_k(0, 3),
                        7: g_q(0, 1), 9: g_r(0), 11: g_r(1), 13: g_r(2),
                        15: g_r(3)}
                fillers = [
                    g_q(0, 2), g_r(4), g_r(5), g_r(6), g_r(7), g_q(0, 3),
                    g_k(1, 0), g_k(1, 1),
                    g_k(1, 2), g_k(1, 3), g_q(1, 0), g_r(8), g_r(9),
                    g_r(10), g_r(11), g_q(1, 1),
                    g_q(1, 2), g_q(1, 3), g_r(12), g_r(13), g_r(14),
                    g_r(15),
                ]
                fillers.reverse()   # consumed via pop()

                cc_in = dram.tile([NS, 1024], F32)
                cc_out = dram.tile([NS, GROUPS * 1024], F32)
                cc_in23 = dram.tile([1, 2048], F32)   # qp-major: qp*16+(qn-2)*8+v*4+qt
                cc_out23 = dram.tile([1, GROUPS * 2048], F32)
                out_t = out_d.rearrange("(t p) c -> p t c", p=128)

                # ---------------- pipeline stages ----------------
                def s1a_xupdate(hp, qn, ctxc, den_r):
                    """x[q, head dims] += ctx * recip(denom); per-partition
                    scalars only. Deferred one block; pure DVE."""
                    for h2 in range(2):
                        g = 2 * hp + h2
                        for qt in range(4):
                            tmp = stg.tile([128, HD], F32, name="tmp")
                            nc.vector.tensor_scalar_mul(
                                out=tmp,
                                in0=ctxc[:, h2, qt * 128:qt * 128 + HD],
                                scalar1=den_r[:, h2, qt:qt + 1])
                            xs = x[:, 4 * qn + qt, g * HD:(g + 1) * HD]
                            nc.vector.tensor_tensor(
                                out=xs, in0=xs, in1=tmp, op=ALU.add)
                    if hp != MT - 1:
                        return None
                    xq = x[:, 4 * qn:4 * qn + 4, :]
                    xr = xrp.tile([128, 4, DC], F32, name="xr")
                    nc.vector.tensor_scalar_max(out=xr, in0=xq, scalar1=0.0)
                    x2 = xrp.tile([128, 4, DC], F32, name="x2")
                    nc.vector.tensor_tensor(
                        out=x2, in0=xr, in1=xr, op=ALU.mult)
                    return xr, x2

                def s1b_stats(qn, xr, x2, do_ag=True):
                    """local stats via free-dim reduces + AllGather issue.
                    Chunks 2 and 3 share one merged AllGather at the drain
                    (the two would otherwise serialize on the collective
                    device right at the end)."""
                    st = rows.tile([128, 2, 4], F32, name="st")
                    nc.vector.tensor_reduce(
                        out=st[:, 0, :], in_=xr, axis=mybir.AxisListType.X,
                        op=ALU.add)
                    nc.vector.tensor_reduce(
                        out=st[:, 1, :], in_=x2, axis=mybir.AxisListType.X,
                        op=ALU.add)
                    if not do_ag:
                        # qp-major staging for the merged chunk-2/3 gather
                        nc.sync.dma_start(
                            out=bass.AP(tensor=cc_in23.tensor,
                                        offset=cc_in23.offset + (qn - 2) * 8,
                                        ap=[[16, 128], [1, 8]]),
                            in_=st)
                        return
                    # dram layout per rank: [kind v][q = qt*128 + qp]
                    nc.sync.dma_start(
                        out=bass.AP(tensor=cc_in.tensor,
                                    offset=cc_in.offset + qn * 1024,
                                    ap=[[1, 128], [512, 2], [128, 4]]),
                        in_=st)
                    if do_ag:
                        nc.gpsimd.collective_compute(
                            "AllGather", ALU.bypass,
                            replica_groups=[[0, 1, 2, 3], [4, 5, 6, 7]],
                            ins=[cc_in[qn:qn + 1, :].opt()],
                            outs=[cc_out[qn:qn + 1, :].opt()],
                        )

                def s2_rowmath(qn, dmae=None, merged=False):
                    """group stats -> per-query A (rstd), B (mean*rstd)."""
                    dmae = dmae or nc.sync
                    if merged:
                        # rank stride 2048 != contiguous -> keep 3-dim APs
                        # on both sides (pad dest stride to 9 so it can't
                        # auto-merge)
                        rsb = rows.tile([128, GROUPS, 8], F32, name="rsb")
                        src_ap = bass.AP(
                            tensor=cc_out23.tensor,
                            offset=cc_out23.offset + (qn - 2) * 8,
                            ap=[[16, 128], [2048, GROUPS], [1, 8]])
                        dmae.dma_start(out=rsb, in_=src_ap)
                        rs = [rsb[:, r] for r in range(GROUPS)]
                    else:
                        rsb = rows.tile([128, GROUPS, 2, 4], F32, name="rsb")
                        src_ap = bass.AP(
                            tensor=cc_out.tensor,
                            offset=cc_out.offset + qn * GROUPS * 1024,
                            ap=[[1, 128], [1024, GROUPS], [512, 2],
                                [128, 4]])
                        dmae.dma_start(out=rsb, in_=src_ap)
                        rs = [rsb[:, r].rearrange("p a b -> p (a b)")
                              for r in range(GROUPS)]
                    acc = rows.tile([128, 8], F32, name="acc")
                    nc.vector.tensor_tensor(
                        out=acc, in0=rs[0], in1=rs[1], op=ALU.add)
                    nc.vector.tensor_tensor(
                        out=acc, in0=acc, in1=rs[2], op=ALU.add)
                    nc.vector.tensor_tensor(
                        out=acc, in0=acc, in1=rs[3], op=ALU.add)
                    mm = rows.tile([128, 8], F32, name="mm")
                    nc.vector.tensor_scalar_mul(
                        out=mm, in0=acc, scalar1=1.0 / D)
                    var = rows.tile([128, 4], F32, name="var")
                    nc.vector.tensor_tensor(
                        out=var, in0=mm[:, 0:4], in1=mm[:, 0:4],
                        op=ALU.mult)
                    nc.vector.tensor_tensor(
                        out=var, in0=mm[:, 4:8], in1=var, op=ALU.subtract)
                    sd = rows.tile([128, 4], F32, name="sd")
                    nc.scalar.activation(
                        out=sd, in_=var, func=AF.Sqrt, bias=eps_c)
                    AB = rows.tile([128, 2, 4], F32, name="AB")
                    nc.vector.reciprocal(out=AB[:, 0, :], in_=sd)
                    nc.vector.tensor_tensor(
                        out=AB[:, 1, :], in0=mm[:, 0:4], in1=AB[:, 0, :],
                        op=ALU.mult)
                    return AB

                def s3_apply(qn, AB, dmae=None, pool=False):
                    """out = (relu(x)*A - B)*gamma + beta, DMA out."""
                    dmae = dmae or nc.sync
                    for qt4 in range(4):
                        qt = 4 * qn + qt4
                        eng = nc.gpsimd if (pool and qt4 % 2) else nc.vector
                        y = stg.tile([128, DC], F32, name="y")
                        eng.tensor_scalar(
                            out=y, in0=x[:, qt, :],
                            scalar1=0.0, scalar2=AB[:, 0, qt4:qt4 + 1],
                            op0=ALU.max, op1=ALU.mult)
                        eng.tensor_scalar(
                            out=y, in0=y, scalar1=AB[:, 1, qt4:qt4 + 1],
                            scalar2=None, op0=ALU.subtract)
                        eng.tensor_tensor(
                            out=y, in0=y, in1=gm_b, op=ALU.mult)
                        eng.tensor_tensor(
                            out=x[:, qt, :], in0=y, in1=bt_b, op=ALU.add)
                        if qt4 == 1:
                            dmae.dma_start(
                                out=out_t[:, 4 * qn:4 * qn + 2, :],
                                in_=x[:, 4 * qn:4 * qn + 2, :])
                    dmae.dma_start(
                        out=out_t[:, 4 * qn + 2:4 * qn + 4, :],
                        in_=x[:, 4 * qn + 2:4 * qn + 4, :])

                pend_xu = []      # (hp, qn, ctxc, den_r)
                pend_st = []      # (qn, xr, x2)
                pend_ag = []      # (qn, issue_block)
                pend_s3 = []      # (qn, AB)
                bi = 0

                def do_s1a():
                    if pend_xu:
                        hp_, qn_, ctxc_, den_ = pend_xu.pop(0)
                        r = s1a_xupdate(hp_, qn_, ctxc_, den_)
                        if r is not None:
                            pend_st.append((qn_, r[0], r[1]))

                def do_s1b(bi):
                    if pend_st:
                        qn_, xr_, x2_ = pend_st.pop(0)
                        s1b_stats(qn_, xr_, x2_, do_ag=True)
                        pend_ag.append((qn_, bi))

                def do_s2(bi, min_age=1, dmae=None):
                    if pend_ag and bi - pend_ag[0][1] >= min_age:
                        qn_, _ = pend_ag.pop(0)
                        pend_s3.append((qn_, s2_rowmath(qn_, dmae)))

                # ================= attention =================
                BLOCKS = [(0, 0), (0, 1), (0, 2), (0, 3),
                          (1, 0), (1, 1), (1, 2), (1, 3)]
                if True:
                    for hp, qn in BLOCKS:
                        qs = slice(qn * 512, (qn + 1) * 512)
                        # one PSUM bank hosts 4 accumulation groups (one
                        # per query tile): matmul start=True zeroing is
                        # bank-granular, so pre-zero via DVE and accumulate
                        # with start=False throughout.
                        ctxA = ctxps.tile([128, 512], F32, name="ctx")
                        ctxB = ctxps.tile([128, 512], F32, name="ctx")
                        nc.vector.memset(ctxA, 0.0)
                        nc.vector.memset(ctxB, 0.0)

                        def ctx_mms(pt, kp):
                            for h2, cps in ((0, ctxA), (1, ctxB)):
                                for qt4 in range(4):
                                    nc.tensor.matmul(
                                        out=cps[:, qt4 * 128:qt4 * 128 + HD + 1],
                                        lhsT=pt[:, :, h2,
                                                qt4 * 128:(qt4 + 1) * 128],
                                        rhs=vA[:, kp, :, 2 * hp + h2,
                                               0:HD + 1],
                                        start=False, stop=(kp == KP - 1),
                                        perf_mode=DR)

                        prev = None
                        for kp in range(KP):
                            pt = ptp.tile([128, 2, 2, 512], F8E5, name="pt")
                            for par in range(2):
                                ks = 2 * kp + par
                                sc = scps.tile([128, 1024], F32, name="sc")
                                kslc = slice(ks * 128, (ks + 1) * 128)
                                nc.tensor.matmul(
                                    out=sc[:, 0:512],
                                    lhsT=kT[0:64, hp, kslc],
                                    rhs=qT[0:64, hp, qs])
                                nc.tensor.matmul(
                                    out=sc[:, 512:1024],
                                    lhsT=kT[64:128, hp, kslc],
                                    rhs=qT[64:128, hp, qs])
                                nc.scalar.activation(
                                    out=pt[:, par], in_=sc, func=AF.Exp,
                                    scale=float(1.0 / np.sqrt(HD)))
                                if hp == 0 and qn == 0:
                                    v_group(ks)
                                    if ks in blk0:
                                        blk0[ks]()
                                elif fillers and ks % 2 == 0:
                                    fillers.pop()()
                            if prev is not None:
                                ctx_mms(*prev)
                                if kp == 1:
                                    do_s1a()
                                elif kp == 2:
                                    do_s1b(bi)
                                elif kp == 4:
                                    do_s2(bi)
                            prev = (pt, kp)
                        ctx_mms(*prev)

                        # denominators (per-partition!) + ctx copy-out
                        den_r = small.tile([128, 2, 4], F32, name="den")
                        for h2, cps in ((0, ctxA), (1, ctxB)):
                            nc.vector.reciprocal(
                                out=den_r[:, h2, :],
                                in_=bass.AP(tensor=cps.tensor,
                                            offset=cps.offset + HD,
                                            ap=[list(cps.ap[0]), [128, 4]]))
                        ctxc = stg.tile([128, 2, 512], F32, name="ctxc")
                        nc.vector.tensor_copy(out=ctxc[:, 0, :], in_=ctxA)
                        nc.vector.tensor_copy(out=ctxc[:, 1, :], in_=ctxB)
                        pend_xu.append((hp, qn, ctxc, den_r))
                        if pend_s3:
                            s3_apply(*pend_s3.pop(0))
                        bi += 1

                # drain
                do_s1a()
                do_s1b(bi)
                do_s2(bi, min_age=0)                   # qn=2
                do_s2(bi, min_age=0, dmae=nc.scalar)   # qn=3 (waits AG(3))
                s3_apply(*pend_s3.pop(0), pool=True)   # qn=2
                s3_apply(*pend_s3.pop(0), dmae=nc.scalar, pool=True)
            p1sb_cm.__exit__(None, None, None)
    _split_waits(nc)
    return nc


_NC = None
LAST_RESULT = None


def _get_nc():
    global _NC
    if _NC is None:
        _NC = build_bass()
    return _NC


def kernel(hidden_states, Wq, bq, Wk, bk, Wv, bv, Wp, gamma, beta):
    hs = np.asarray(hidden_states, dtype=np.float32)
    Wq = np.asarray(Wq, np.float32)
    Wk = np.asarray(Wk, np.float32)
    Wv = np.asarray(Wv, np.float32)
    Wp = np.asarray(Wp, np.float32)
    bq = np.asarray(bq, np.float32)
    bk = np.asarray(bk, np.float32)
    bv = np.asarray(bv, np.float32)
    gamma = np.asarray(gamma, np.float32)
    beta = np.asarray(beta, np.float32)
    bf = ml_dtypes.bfloat16

    nc = _get_nc()
    in_maps = []
    for c in range(NCORES):
        b, g = divmod(c, GROUPS)
        sl = slice(g * DC, (g + 1) * DC)
        in_maps.append({
            "hsT": np.ascontiguousarray(hs[b].T.astype(bf)),
            "hsT8": np.ascontiguousarray(
                hs[b].T.astype(ml_dtypes.float8_e4m3)).view(np.uint8),
            "wvT8": np.ascontiguousarray(
                Wv[sl].T.astype(ml_dtypes.float8_e4m3)).view(np.uint8),
            "wqT": np.ascontiguousarray(Wq[sl].T.astype(bf)),
            "wkT": np.ascontiguousarray(Wk[sl].T.astype(bf)),
            "wpT": np.ascontiguousarray(Wp[sl].T.astype(bf)),
            "bq": np.ascontiguousarray(bq[sl]),
            "bk": np.ascontiguousarray(bk[sl]),
            "bv": np.ascontiguousarray(bv[sl]),
            "gamma": np.ascontiguousarray(gamma[sl]),
            "beta": np.ascontiguousarray(beta[sl]),
        })
    res = run_bass_kernel_spmd(nc, in_maps, core_ids=list(range(NCORES)))
    global LAST_RESULT
    LAST_RESULT = res
    out = np.empty((B, S, D), np.float32)
    for c, r in enumerate(res.results):
        b, g = divmod(c, GROUPS)
        out[b, :, g * DC:(g + 1) * DC] = r["out"]
    return out


# revision 8
# speedup vs baseline: 1.0772x; 1.0189x over previous
"""Trainium2 Bass kernel for nn_BertAttention_78554951843978 (v3, q-major ctx).

Sharding: data-parallel over B (2 groups of 4 cores), tensor-parallel over
D within a group (256 dims = 4 heads per core).

Key structure:
  - hsT/weights in bf16; qT/kT bf16 (d-major, for scores)
  - probs fp8e5 from the exp activation; V fp8e4 in DoubleRow layout;
    ctx matmuls run fp8 DoubleRow TRANSPOSED: out[q, v-dim] with q on
    partitions, so the softmax denominator (ones column) and the
    layernorm A/B terms are all per-partition scalars -> no cross-
    partition broadcast bounces at all
  - x kept q-major [q % 128, qtile, D-slice]; residual projection done
    natural-layout (hs-stationary) straight into x
  - layernorm stats are DVE reduces (free-dim!) per query chunk,
    AllGather'ed per chunk (1.0x collective cost vs AllReduce's 1.875x),
    reduced locally, applied per chunk, pipelined behind the attention
  - PE warm-up spin at t=0 so the first real matmuls run at full clock
"""

import numpy as np
import ml_dtypes

import concourse.bass as bass
import concourse.tile as tile
from concourse import mybir
from concourse.bass_utils import run_bass_kernel_spmd

B, S, D, H = 2, 2048, 1024, 16
HD = 64
NCORES = 8
GROUPS = 4          # cores per batch
DC = D // GROUPS    # 256 dims per core
EPS = 1e-12
MP = 80             # V slot: 64 dims + 1 ones + pad to 16B slot stride

F32 = mybir.dt.float32
F32R = mybir.dt.float32r
BF16 = mybir.dt.bfloat16
F8E4 = mybir.dt.float8e4
F8E5 = mybir.dt.float8e5
AF = mybir.ActivationFunctionType
DR = mybir.MatmulPerfMode.DoubleRow
ALU = mybir.AluOpType

KT = D // 128    # 8 contraction tiles
MT = DC // 128   # 2 head pairs
NS = S // 512    # 4 query chunks of 512
ST = S // 128    # 16 key tiles
KP = ST // 2     # 8 key-tile pairs (DoubleRow)
QT = S // 128    # 16 query tiles of 128


def _split_waits(nc, keep=1):
    """Walrus rejects >1 sem wait per (non-EVSEM) instruction; hoist extras
    onto preceding single-wait NOPs on the same engine."""
    for bb in nc.main_func.blocks:
        insts = list(bb.instructions)
        out_list = []
        changed = False
        for inst in insts:
            si = inst.sync_info
            cap = 2 if isinstance(inst, mybir.InstEventSemaphore) else keep
            if si is not None and si.on_wait is not None and len(si.on_wait) > cap:
                waits = list(si.on_wait)
                for w in waits[cap:]:
                    out_list.append(mybir.InstNoOp(
                        name=nc.get_next_instruction_name(),
                        engine=inst.engine,
                        ins=[], outs=[],
                        sync_info=mybir.SyncInfo(on_wait=[w], on_update=[]),
                        bass_nofuse=True,
                    ))
                inst.sync_info = mybir.SyncInfo(
                    on_wait=waits[:cap], on_update=list(si.on_update or []))
                changed = True
            out_list.append(inst)
        if changed:
            bb.instructions = out_list


def build_bass():
    nc = bass.Bass(num_devices=NCORES)

    # ---------------- DRAM I/O ----------------
    hsT_d = nc.dram_tensor("hsT", [D, S], BF16, kind="ExternalInput")
    hsT8_d = nc.dram_tensor("hsT8", [D, S], F8E4, kind="ExternalInput")
    wvT8_d = nc.dram_tensor("wvT8", [D, DC], F8E4, kind="ExternalInput")
    wqT_d = nc.dram_tensor("wqT", [D, DC], F8E4, kind="ExternalInput")
    wkT_d = nc.dram_tensor("wkT", [D, DC], F8E4, kind="ExternalInput")
    wpT_d = nc.dram_tensor("wpT", [D, DC], BF16, kind="ExternalInput")
    bq_d = nc.dram_tensor("bq", [DC], F32, kind="ExternalInput")
    bk_d = nc.dram_tensor("bk", [DC], F32, kind="ExternalInput")
    bv_d = nc.dram_tensor("bv", [DC], F32, kind="ExternalInput")
    gm_d = nc.dram_tensor("gamma", [DC], F32, kind="ExternalInput")
    bt_d = nc.dram_tensor("beta", [DC], F32, kind="ExternalInput")
    out_d = nc.dram_tensor("out", [S, DC], F32, kind="ExternalOutput")

    with tile.TileContext(nc) as tc:
        with (
            tc.tile_pool(name="persist", bufs=1) as persist,
            tc.tile_pool(name="dram", bufs=1, space="DRAM") as dram,
        ):
            # ------------- persistent SBUF -------------
            qT = persist.tile([128, MT, S], BF16)            # 8 KB/part
            kT = persist.tile([128, MT, S], BF16)
            x = persist.tile([128, QT, DC], F32)             # q-major, 16 KB
            # DoubleRow-packed aug V: [kp][parity][head g][MP]
            vA = persist.tile([128, KP, 2, GROUPS, MP], F8E4)
            # small constants: cols 0..3 = bq|bk (2 each), 4 = f32r ones,
            # 5 = eps, 6..261 = bv bcast, 262..517 gamma bcast, 518..773 beta
            cst = persist.tile([128, 6 + 3 * DC], F32)
            bq_s, bk_s = cst[:, 0:2], cst[:, 2:4]
            eps_c = cst[:, 5:6]
            bv_b = cst[:, 6:6 + DC]
            gm_b = cst[:, 6 + DC:6 + 2 * DC]
            bt_b = cst[:, 6 + 2 * DC:6 + 3 * DC]
            wsrc = persist.tile([128, 256], BF16)            # warm-up fodder
            onesr = persist.tile([128, 1], BF16)

            p1sb_cm = tc.tile_pool(name="p1sb", bufs=1)
            p1sb = p1sb_cm.__enter__()
            hsT = p1sb.tile([128, KT, S], BF16)              # 32 KB/part
            hsT8 = p1sb.tile([128, KT, S], F8E4)             # 16 KB/part
            wv8 = p1sb.tile([128, KT, DC], F8E4)
            # (bf16 wv no longer needed: V runs on the fp8 path)
            wq = p1sb.tile([128, KT, MT, 128], F8E4)
            wk = p1sb.tile([128, KT, MT, 128], F8E4)
            wp = p1sb.tile([128, KT, DC], BF16)

            # input DMAs: few, large transfers (desc-gen is serialized);
            # first-needed slices first; scalar queue untouched.
            hsT_t = hsT_d.rearrange("(t p) s -> p t s", p=128)
            wq_t = wqT_d.rearrange("(t p) (m f) -> p t m f", p=128, f=128)
            wk_t = wkT_d.rearrange("(t p) (m f) -> p t m f", p=128, f=128)
            wp_t = wpT_d.rearrange("(t p) c -> p t c", p=128)
            nc.gpsimd.dma_start(out=bq_s, in_=bq_d.rearrange("(m p) -> p m", p=128))
            nc.gpsimd.dma_start(out=bk_s, in_=bk_d.rearrange("(m p) -> p m", p=128))
            nc.sync.dma_start(out=wq, in_=wq_t)
            for k in range(KT):
                e = nc.sync if k % 2 == 0 else nc.gpsimd
                e.dma_start(out=hsT[:, k, 0:512], in_=hsT_t[:, k, 0:512])
            nc.gpsimd.dma_start(out=wk, in_=wk_t)
            nc.gpsimd.dma_start(out=bv_b, in_=bass.AP(
                tensor=bv_d[:].tensor, offset=0, ap=[[0, 128], [1, DC]]))
            nc.sync.dma_start(
                out=wv8, in_=wvT8_d.rearrange("(t p) c -> p t c", p=128))
            hsT8_t = hsT8_d.rearrange("(t p) s -> p t s", p=128)
            for k in range(KT):
                e = nc.sync if k % 2 == 0 else nc.gpsimd
                e.dma_start(out=hsT8[:, k, 0:512], in_=hsT8_t[:, k, 0:512])
                e.dma_start(out=hsT[:, k, 512:1024], in_=hsT_t[:, k, 512:1024])
            for k in range(KT):
                e = nc.sync if k % 2 == 0 else nc.gpsimd
                e.dma_start(out=hsT8[:, k, 512:2048], in_=hsT8_t[:, k, 512:2048])
            for k in range(KT):
                e = nc.sync if k % 2 == 0 else nc.gpsimd
                e.dma_start(out=hsT[:, k, 1024:2048],
                            in_=hsT_t[:, k, 1024:2048])
            nc.gpsimd.dma_start(out=wp, in_=wp_t)
            nc.gpsimd.dma_start(out=gm_b, in_=bass.AP(
                tensor=gm_d[:].tensor, offset=0, ap=[[0, 128], [1, DC]]))
            nc.gpsimd.dma_start(out=bt_b, in_=bass.AP(
                tensor=bt_d[:].tensor, offset=0, ap=[[0, 128], [1, DC]]))
            nc.vector.memset(onesr, 1.0)
            nc.vector.memset(eps_c, EPS)
            nc.vector.memset(wsrc, 1.0)
            nc.vector.memset(vA[:, :, :, :, HD:MP], 0.0)
            nc.vector.memset(vA[:, :, :, :, HD:HD + 1], 1.0)

            with (
                tc.tile_pool(name="pps", bufs=2, space="PSUM") as pps,
                tc.tile_pool(name="scps", bufs=2, space="PSUM") as scps,
                tc.tile_pool(name="ctxps", bufs=2, space="PSUM") as ctxps,
                tc.tile_pool(name="ptp", bufs=4) as ptp,
                tc.tile_pool(name="small", bufs=2) as small,
                tc.tile_pool(name="stg", bufs=2) as stg,
                tc.tile_pool(name="xrp", bufs=2) as xrp,
                tc.tile_pool(name="rows", bufs=2) as rows,
            ):
                # PE warm-up: ~28 cheap matmuls so pe ramps to full clock
                # before the first real projection (which waits on DMAs).
                wps = pps.tile([128, 512], F32, name="gps")
                for i in range(24):
                    nc.tensor.matmul(out=wps[0:1, 0:256], lhsT=onesr,
                                     rhs=wsrc, start=True, stop=True)

                def proj_group(w_sb, m, n, bias, out_sb):
                    """One [128,512] q/k projection block, fp8 DoubleRow
                    (scores are softmax-diluted; fp8 error is negligible
                    downstream)."""
                    ps = pps.tile([128, 512], F32, name="gps")
                    for t in range(KT // 2):
                        nc.tensor.matmul(
                            out=ps,
                            lhsT=w_sb[:, 2 * t:2 * t + 2, m, :],
                            rhs=hsT8[:, 2 * t:2 * t + 2,
                                     n * 512:(n + 1) * 512],
                            start=(t == 0), stop=(t == KT // 2 - 1),
                            perf_mode=DR)
                    o = out_sb[:, m, n * 512:(n + 1) * 512]
                    nc.vector.tensor_scalar_add(out=o, in0=ps, scalar1=bias)

                def v_group(j):
                    """V for key tile j via fp8 DoubleRow (4x fewer PE
                    cycles; V is quantized to fp8e4 downstream anyway)."""
                    kp, par = divmod(j, 2)
                    ps = pps.tile([128, 512], F32, name="gps")
                    for t in range(KT // 2):
                        nc.tensor.matmul(
                            out=ps[:, 0:DC],
                            lhsT=hsT8[:, 2 * t:2 * t + 2,
                                      j * 128:(j + 1) * 128],
                            rhs=wv8[:, 2 * t:2 * t + 2, :],
                            start=(t == 0), stop=(t == KT // 2 - 1),
                            perf_mode=DR)
                    nc.vector.tensor_add(
                        out=vA[:, kp, par, :, 0:HD],
                        in0=ps[:, 0:DC].rearrange("p (h d) -> p h d", d=HD),
                        in1=bv_b.rearrange("p (h d) -> p h d", d=HD))

                def r_group(qt):
                    """Residual hs@Wp.T for query tile qt, natural layout,
                    straight into q-major x."""
                    ps = pps.tile([128, 512], F32, name="gps")
                    for k in range(KT):
                        nc.tensor.matmul(
                            out=ps[:, 0:DC],
                            lhsT=hsT[:, k, qt * 128:(qt + 1) * 128],
                            rhs=wp[:, k, :],
                            start=(k == 0), stop=(k == KT - 1))
                    nc.vector.tensor_copy(out=x[:, qt, :], in_=ps[:, 0:DC])

                def g_q(m, n):
                    return lambda: proj_group(wq, m, n, bq_s[:, m:m + 1], qT)

                def g_k(m, n):
                    return lambda: proj_group(wk, m, n, bk_s[:, m:m + 1], kT)

                def g_r(qt):
                    return lambda: r_group(qt)

                for g in [g_q(0, 0), g_k(0, 0)]:
                    g()
                blk0 = {1: g_k(0, 1), 3: g_k(0, 2), 5: g_k(0, 3),
                        7: g_q(0, 1), 9: g_r(0), 11: g_r(1), 13: g_r(2),
                        15: g_r(3)}
                fillers = [
                    g_q(0, 2), g_r(4), g_r(5), g_r(6), g_r(7), g_q(0, 3),
                    g_k(1, 0), g_k(1, 1),
                    g_k(1, 2), g_k(1, 3), g_q(1, 0), g_r(8), g_r(9),
                    g_r(10), g_r(11), g_q(1, 1),
                    g_q(1, 2), g_q(1, 3), g_r(12), g_r(13), g_r(14),
                    g_r(15),
                ]
                fillers.reverse()   # consumed via pop()

                cc_in = dram.tile([NS, 1024], F32)
                cc_out = dram.tile([NS, GROUPS * 1024], F32)
                cc_in23 = dram.tile([1, 2048], F32)   # qp-major: qp*16+(qn-2)*8+v*4+qt
                cc_out23 = dram.tile([1, GROUPS * 2048], F32)
                out_t = out_d.rearrange("(t p) c -> p t c", p=128)

                # ---------------- pipeline stages ----------------
                def s1a_xupdate(hp, qn, ctxc, den_r):
                    """x[q, head dims] += ctx * recip(denom); per-partition
                    scalars only. Deferred one block; pure DVE."""
                    for h2 in range(2):
                        g = 2 * hp + h2
                        for qt in range(4):
                            tmp = stg.tile([128, HD], F32, name="tmp")
                            nc.vector.tensor_scalar_mul(
                                out=tmp,
                                in0=ctxc[:, h2, qt * 128:qt * 128 + HD],
                                scalar1=den_r[:, h2, qt:qt + 1])
                            xs = x[:, 4 * qn + qt, g * HD:(g + 1) * HD]
                            nc.vector.tensor_tensor(
                                out=xs, in0=xs, in1=tmp, op=ALU.add)
                    if hp != MT - 1:
                        return None
                    xq = x[:, 4 * qn:4 * qn + 4, :]
                    xr = xrp.tile([128, 4, DC], F32, name="xr")
                    nc.vector.tensor_scalar_max(out=xr, in0=xq, scalar1=0.0)
                    x2 = xrp.tile([128, 4, DC], F32, name="x2")
                    nc.vector.tensor_tensor(
                        out=x2, in0=xr, in1=xr, op=ALU.mult)
                    return xr, x2

                def s1b_stats(qn, xr, x2, do_ag=True):
                    """local stats via free-dim reduces + AllGather issue.
                    Chunks 2 and 3 share one merged AllGather at the drain
                    (the two would otherwise serialize on the collective
                    device right at the end)."""
                    st = rows.tile([128, 2, 4], F32, name="st")
                    nc.vector.tensor_reduce(
                        out=st[:, 0, :], in_=xr, axis=mybir.AxisListType.X,
                        op=ALU.add)
                    nc.vector.tensor_reduce(
                        out=st[:, 1, :], in_=x2, axis=mybir.AxisListType.X,
                        op=ALU.add)
                    if not do_ag:
                        # qp-major staging for the merged chunk-2/3 gather
                        nc.sync.dma_start(
                            out=bass.AP(tensor=cc_in23.tensor,
                                        offset=cc_in23.offset + (qn - 2) * 8,
                                        ap=[[16, 128], [1, 8]]),
                            in_=st)
                        return
                    # dram layout per rank: [kind v][q = qt*128 + qp]
                    nc.sync.dma_start(
                        out=bass.AP(tensor=cc_in.tensor,
                                    offset=cc_in.offset + qn * 1024,
                                    ap=[[1, 128], [512, 2], [128, 4]]),
                        in_=st)
                    if do_ag:
                        nc.gpsimd.collective_compute(
                            "AllGather", ALU.bypass,
                            replica_groups=[[0, 1, 2, 3], [4, 5, 6, 7]],
                            ins=[cc_in[qn:qn + 1, :].opt()],
                            outs=[cc_out[qn:qn + 1, :].opt()],
                        )

                def s2_rowmath(qn, dmae=None, merged=False):
                    """group stats -> per-query A (rstd), B (mean*rstd)."""
                    dmae = dmae or nc.sync
                    if merged:
                        # rank stride 2048 != contiguous -> keep 3-dim APs
                        # on both sides (pad dest stride to 9 so it can't
                        # auto-merge)
                        rsb = rows.tile([128, GROUPS, 8], F32, name="rsb")
                        src_ap = bass.AP(
                            tensor=cc_out23.tensor,
                            offset=cc_out23.offset + (qn - 2) * 8,
                            ap=[[16, 128], [2048, GROUPS], [1, 8]])
                        dmae.dma_start(out=rsb, in_=src_ap)
                        rs = [rsb[:, r] for r in range(GROUPS)]
                    else:
                        rsb = rows.tile([128, GROUPS, 2, 4], F32, name="rsb")
                        src_ap = bass.AP(
                            tensor=cc_out.tensor,
                            offset=cc_out.offset + qn * GROUPS * 1024,
                            ap=[[1, 128], [1024, GROUPS], [512, 2],
                                [128, 4]])
                        dmae.dma_start(out=rsb, in_=src_ap)
                        rs = [rsb[:, r].rearrange("p a b -> p (a b)")
                              for r in range(GROUPS)]
                    acc = rows.tile([128, 8], F32, name="acc")
                    nc.vector.tensor_tensor(
                        out=acc, in0=rs[0], in1=rs[1], op=ALU.add)
                    nc.vector.tensor_tensor(
                        out=acc, in0=acc, in1=rs[2], op=ALU.add)
                    nc.vector.tensor_tensor(
                        out=acc, in0=acc, in1=rs[3], op=ALU.add)
                    mm = rows.tile([128, 8], F32, name="mm")
                    nc.vector.tensor_scalar_mul(
                        out=mm, in0=acc, scalar1=1.0 / D)
                    var = rows.tile([128, 4], F32, name="var")
                    nc.vector.tensor_tensor(
                        out=var, in0=mm[:, 0:4], in1=mm[:, 0:4],
                        op=ALU.mult)
                    nc.vector.tensor_tensor(
                        out=var, in0=mm[:, 4:8], in1=var, op=ALU.subtract)
                    sd = rows.tile([128, 4], F32, name="sd")
                    nc.scalar.activation(
                        out=sd, in_=var, func=AF.Sqrt, bias=eps_c)
                    AB = rows.tile([128, 2, 4], F32, name="AB")
                    nc.vector.reciprocal(out=AB[:, 0, :], in_=sd)
                    nc.vector.tensor_tensor(
                        out=AB[:, 1, :], in0=mm[:, 0:4], in1=AB[:, 0, :],
                        op=ALU.mult)
                    return AB

                def s3_apply(qn, AB, dmae=None, pool=False):
                    """out = (relu(x)*A - B)*gamma + beta, DMA out."""
                    dmae = dmae or nc.sync
                    for qt4 in range(4):
                        qt = 4 * qn + qt4
                        eng = nc.gpsimd if (pool and qt4 % 2) else nc.vector
                        y = stg.tile([128, DC], F32, name="y")
                        eng.tensor_scalar(
                            out=y, in0=x[:, qt, :],
                            scalar1=0.0, scalar2=AB[:, 0, qt4:qt4 + 1],
                            op0=ALU.max, op1=ALU.mult)
                        eng.tensor_scalar(
                            out=y, in0=y, scalar1=AB[:, 1, qt4:qt4 + 1],
                            scalar2=None, op0=ALU.subtract)
                        eng.tensor_tensor(
                            out=y, in0=y, in1=gm_b, op=ALU.mult)
                        eng.tensor_tensor(
                            out=x[:, qt, :], in0=y, in1=bt_b, op=ALU.add)
                        if qt4 == 1:
                            dmae.dma_start(
                                out=out_t[:, 4 * qn:4 * qn + 2, :],
                                in_=x[:, 4 * qn:4 * qn + 2, :])
                    dmae.dma_start(
                        out=out_t[:, 4 * qn + 2:4 * qn + 4, :],
                        in_=x[:, 4 * qn + 2:4 * qn + 4, :])

                pend_xu = []      # (hp, qn, ctxc, den_r)
                pend_st = []      # (qn, xr, x2)
                pend_ag = []      # (qn, issue_block)
                pend_s3 = []      # (qn, AB)
                bi = 0

                def do_s1a():
                    if pend_xu:
                        hp_, qn_, ctxc_, den_ = pend_xu.pop(0)
                        r = s1a_xupdate(hp_, qn_, ctxc_, den_)
                        if r is not None:
                            pend_st.append((qn_, r[0], r[1]))

                def do_s1b(bi):
                    if pend_st:
                        qn_, xr_, x2_ = pend_st.pop(0)
                        s1b_stats(qn_, xr_, x2_, do_ag=True)
                        pend_ag.append((qn_, bi))

                def do_s2(bi, min_age=1, dmae=None):
                    if pend_ag and bi - pend_ag[0][1] >= min_age:
                        qn_, _ = pend_ag.pop(0)
                        pend_s3.append((qn_, s2_rowmath(qn_, dmae)))

                # ================= attention =================
                BLOCKS = [(0, 0), (0, 1), (0, 2), (0, 3),
                          (1, 0), (1, 1), (1, 2), (1, 3)]
                if True:
                    for hp, qn in BLOCKS:
                        qs = slice(qn * 512, (qn + 1) * 512)
                        # one PSUM bank hosts 4 accumulation groups (one
                        # per query tile): matmul start=True zeroing is
                        # bank-granular, so pre-zero via DVE and accumulate
                        # with start=False throughout.
                        ctxA = ctxps.tile([128, 512], F32, name="ctx")
                        ctxB = ctxps.tile([128, 512], F32, name="ctx")
                        nc.vector.memset(ctxA, 0.0)
                        nc.vector.memset(ctxB, 0.0)

                        def ctx_mms(pt, kp):
                            for h2, cps in ((0, ctxA), (1, ctxB)):
                                for qt4 in range(4):
                                    nc.tensor.matmul(
                                        out=cps[:, qt4 * 128:qt4 * 128 + HD + 1],
                                        lhsT=pt[:, :, h2,
                                                qt4 * 128:(qt4 + 1) * 128],
                                        rhs=vA[:, kp, :, 2 * hp + h2,
                                               0:HD + 1],
                                        start=False, stop=(kp == KP - 1),
                                        perf_mode=DR)

                        prev = None
                        for kp in range(KP):
                            pt = ptp.tile([128, 2, 2, 512], F8E5, name="pt")
                            for par in range(2):
                                ks = 2 * kp + par
                                sc = scps.tile([128, 1024], F32, name="sc")
                                kslc = slice(ks * 128, (ks + 1) * 128)
                                nc.tensor.matmul(
                                    out=sc[:, 0:512],
                                    lhsT=kT[0:64, hp, kslc],
                                    rhs=qT[0:64, hp, qs])
                                nc.tensor.matmul(
                                    out=sc[:, 512:1024],
                                    lhsT=kT[64:128, hp, kslc],
                                    rhs=qT[64:128, hp, qs])
                                nc.scalar.activation(
                                    out=pt[:, par], in_=sc, func=AF.Exp,
                                    scale=float(1.0 / np.sqrt(HD)))
                                if hp == 0 and qn == 0:
                                    v_group(ks)
                                    if ks in blk0:
                                        blk0[ks]()
                                elif fillers and ks % 2 == 0:
                                    fillers.pop()()
                            if prev is not None:
                                ctx_mms(*prev)
                                if kp == 1:
                                    do_s1a()
                                elif kp == 2:
                                    do_s1b(bi)
                                elif kp == 4:
                                    do_s2(bi)
                            prev = (pt, kp)
                        ctx_mms(*prev)

                        # denominators (per-partition!) + ctx copy-out
                        den_r = small.tile([128, 2, 4], F32, name="den")
                        for h2, cps in ((0, ctxA), (1, ctxB)):
                            nc.vector.reciprocal(
                                out=den_r[:, h2, :],
                                in_=bass.AP(tensor=cps.tensor,
                                            offset=cps.offset + HD,
                                            ap=[list(cps.ap[0]), [128, 4]]))
                        ctxc = stg.tile([128, 2, 512], F32, name="ctxc")
                        nc.vector.tensor_copy(out=ctxc[:, 0, :], in_=ctxA)
                        nc.vector.tensor_copy(out=ctxc[:, 1, :], in_=ctxB)
                        pend_xu.append((hp, qn, ctxc, den_r))
                        if pend_s3:
                            s3_apply(*pend_s3.pop(0))
                        bi += 1

                # drain
                do_s1a()
                do_s1b(bi)
                do_s2(bi, min_age=0)                   # qn=2
                do_s2(bi, min_age=0, dmae=nc.scalar)   # qn=3 (waits AG(3))
                s3_apply(*pend_s3.pop(0), pool=True)   # qn=2
                s3_apply(*pend_s3.pop(0), dmae=nc.scalar, pool=True)
            p1sb_cm.__exit__(None, None, None)
    _split_waits(nc)
    return nc


_NC = None
LAST_RESULT = None


def _get_nc():
    global _NC
    if _NC is None:
        _NC = build_bass()
    return _NC


def kernel(hidden_states, Wq, bq, Wk, bk, Wv, bv, Wp, gamma, beta):
    hs = np.asarray(hidden_states, dtype=np.float32)
    Wq = np.asarray(Wq, np.float32)
    Wk = np.asarray(Wk, np.float32)
    Wv = np.asarray(Wv, np.float32)
    Wp = np.asarray(Wp, np.float32)
    bq = np.asarray(bq, np.float32)
    bk = np.asarray(bk, np.float32)
    bv = np.asarray(bv, np.float32)
    gamma = np.asarray(gamma, np.float32)
    beta = np.asarray(beta, np.float32)
    bf = ml_dtypes.bfloat16

    nc = _get_nc()
    in_maps = []
    for c in range(NCORES):
        b, g = divmod(c, GROUPS)
        sl = slice(g * DC, (g + 1) * DC)
        in_maps.append({
            "hsT": np.ascontiguousarray(hs[b].T.astype(bf)),
            "hsT8": np.ascontiguousarray(
                hs[b].T.astype(ml_dtypes.float8_e4m3)).view(np.uint8),
            "wvT8": np.ascontiguousarray(
                Wv[sl].T.astype(ml_dtypes.float8_e4m3)).view(np.uint8),
            "wqT": np.ascontiguousarray(
                Wq[sl].T.astype(ml_dtypes.float8_e4m3)).view(np.uint8),
            "wkT": np.ascontiguousarray(
                Wk[sl].T.astype(ml_dtypes.float8_e4m3)).view(np.uint8),
            "wpT": np.ascontiguousarray(Wp[sl].T.astype(bf)),
            "bq": np.ascontiguousarray(bq[sl]),
            "bk": np.ascontiguousarray(bk[sl]),
            "bv": np.ascontiguousarray(bv[sl]),
            "gamma": np.ascontiguousarray(gamma[sl]),
            "beta": np.ascontiguousarray(beta[sl]),
        })
    res = run_bass_kernel_spmd(nc, in_maps, core_ids=list(range(NCORES)))
    global LAST_RESULT
    LAST_RESULT = res
    out = np.empty((B, S, D), np.float32)
    for c, r in enumerate(res.results):
        b, g = divmod(c, GROUPS)
        out[b, :, g * DC:(g + 1) * DC] = r["out"]
    return out


# revision 9
# speedup vs baseline: 1.0772x; 1.0000x over previous
"""Trainium2 Bass kernel for nn_BertAttention_78554951843978 (v3, q-major ctx).

Sharding: data-parallel over B (2 groups of 4 cores), tensor-parallel over
D within a group (256 dims = 4 heads per core).

Key structure:
  - hsT/weights in bf16; qT/kT bf16 (d-major, for scores)
  - probs fp8e5 from the exp activation; V fp8e4 in DoubleRow layout;
    ctx matmuls run fp8 DoubleRow TRANSPOSED: out[q, v-dim] with q on
    partitions, so the softmax denominator (ones column) and the
    layernorm A/B terms are all per-partition scalars -> no cross-
    partition broadcast bounces at all
  - x kept q-major [q % 128, qtile, D-slice]; residual projection done
    natural-layout (hs-stationary) straight into x
  - layernorm stats are DVE reduces (free-dim!) per query chunk,
    AllGather'ed per chunk (1.0x collective cost vs AllReduce's 1.875x),
    reduced locally, applied per chunk, pipelined behind the attention
  - PE warm-up spin at t=0 so the first real matmuls run at full clock
"""

import numpy as np
import ml_dtypes

import concourse.bass as bass
import concourse.tile as tile
from concourse import mybir
from concourse.bass_utils import run_bass_kernel_spmd

B, S, D, H = 2, 2048, 1024, 16
HD = 64
NCORES = 8
GROUPS = 4          # cores per batch
DC = D // GROUPS    # 256 dims per core
EPS = 1e-12
MP = 80             # V slot: 64 dims + 1 ones + pad to 16B slot stride

F32 = mybir.dt.float32
F32R = mybir.dt.float32r
BF16 = mybir.dt.bfloat16
F8E4 = mybir.dt.float8e4
F8E5 = mybir.dt.float8e5
AF = mybir.ActivationFunctionType
DR = mybir.MatmulPerfMode.DoubleRow
ALU = mybir.AluOpType

KT = D // 128    # 8 contraction tiles
MT = DC // 128   # 2 head pairs
NS = S // 512    # 4 query chunks of 512
ST = S // 128    # 16 key tiles
KP = ST // 2     # 8 key-tile pairs (DoubleRow)
QT = S // 128    # 16 query tiles of 128


def _split_waits(nc, keep=1):
    """Walrus rejects >1 sem wait per (non-EVSEM) instruction; hoist extras
    onto preceding single-wait NOPs on the same engine."""
    for bb in nc.main_func.blocks:
        insts = list(bb.instructions)
        out_list = []
        changed = False
        for inst in insts:
            si = inst.sync_info
            cap = 2 if isinstance(inst, mybir.InstEventSemaphore) else keep
            if si is not None and si.on_wait is not None and len(si.on_wait) > cap:
                waits = list(si.on_wait)
                for w in waits[cap:]:
                    out_list.append(mybir.InstNoOp(
                        name=nc.get_next_instruction_name(),
                        engine=inst.engine,
                        ins=[], outs=[],
                        sync_info=mybir.SyncInfo(on_wait=[w], on_update=[]),
                        bass_nofuse=True,
                    ))
                inst.sync_info = mybir.SyncInfo(
                    on_wait=waits[:cap], on_update=list(si.on_update or []))
                changed = True
            out_list.append(inst)
        if changed:
            bb.instructions = out_list


def build_bass():
    nc = bass.Bass(num_devices=NCORES)

    # ---------------- DRAM I/O ----------------
    hsT_d = nc.dram_tensor("hsT", [D, S], BF16, kind="ExternalInput")
    hsT8_d = nc.dram_tensor("hsT8", [D, S], F8E4, kind="ExternalInput")
    wvT8_d = nc.dram_tensor("wvT8", [D, DC], F8E4, kind="ExternalInput")
    wqT_d = nc.dram_tensor("wqT", [D, DC], F8E4, kind="ExternalInput")
    wkT_d = nc.dram_tensor("wkT", [D, DC], F8E4, kind="ExternalInput")
    wpT_d = nc.dram_tensor("wpT", [D, DC], BF16, kind="ExternalInput")
    bq_d = nc.dram_tensor("bq", [DC], F32, kind="ExternalInput")
    bk_d = nc.dram_tensor("bk", [DC], F32, kind="ExternalInput")
    bv_d = nc.dram_tensor("bv", [DC], F32, kind="ExternalInput")
    gm_d = nc.dram_tensor("gamma", [DC], F32, kind="ExternalInput")
    bt_d = nc.dram_tensor("beta", [DC], F32, kind="ExternalInput")
    out_d = nc.dram_tensor("out", [S, DC], F32, kind="ExternalOutput")

    with tile.TileContext(nc) as tc:
        with (
            tc.tile_pool(name="persist", bufs=1) as persist,
            tc.tile_pool(name="dram", bufs=1, space="DRAM") as dram,
        ):
            # ------------- persistent SBUF -------------
            qT = persist.tile([128, MT, S], BF16)            # 8 KB/part
            kT = persist.tile([128, MT, S], BF16)
            x = persist.tile([128, QT, DC], F32)             # q-major, 16 KB
            # DoubleRow-packed aug V: [kp][parity][head g][MP]
            vA = persist.tile([128, KP, 2, GROUPS, MP], F8E4)
            # small constants: cols 0..3 = bq|bk (2 each), 4 = f32r ones,
            # 5 = eps, 6..261 = bv bcast, 262..517 gamma bcast, 518..773 beta
            cst = persist.tile([128, 6 + 3 * DC], F32)
            bq_s, bk_s = cst[:, 0:2], cst[:, 2:4]
            eps_c = cst[:, 5:6]
            bv_b = cst[:, 6:6 + DC]
            gm_b = cst[:, 6 + DC:6 + 2 * DC]
            bt_b = cst[:, 6 + 2 * DC:6 + 3 * DC]
            wsrc = persist.tile([128, 256], BF16)            # warm-up fodder
            onesr = persist.tile([128, 1], BF16)

            p1sb_cm = tc.tile_pool(name="p1sb", bufs=1)
            p1sb = p1sb_cm.__enter__()
            hsT = p1sb.tile([128, KT, S], BF16)              # 32 KB/part
            hsT8 = p1sb.tile([128, KT, S], F8E4)             # 16 KB/part
            wv8 = p1sb.tile([128, KT, DC], F8E4)
            # (bf16 wv no longer needed: V runs on the fp8 path)
            wq = p1sb.tile([128, KT, MT, 128], F8E4)
            wk = p1sb.tile([128, KT, MT, 128], F8E4)
            wp = p1sb.tile([128, KT, DC], BF16)

            # input DMAs: few, large transfers (desc-gen is serialized);
            # first-needed slices first; scalar queue untouched.
            hsT_t = hsT_d.rearrange("(t p) s -> p t s", p=128)
            wq_t = wqT_d.rearrange("(t p) (m f) -> p t m f", p=128, f=128)
            wk_t = wkT_d.rearrange("(t p) (m f) -> p t m f", p=128, f=128)
            wp_t = wpT_d.rearrange("(t p) c -> p t c", p=128)
            # fp8 stream first: the opening q/k/v projections all consume
            # hsT8 (half the bytes of bf16); bf16 hsT is only needed by the
            # residual fillers which start ~a block later
            nc.gpsimd.dma_start(out=bq_s, in_=bq_d.rearrange("(m p) -> p m", p=128))
            nc.gpsimd.dma_start(out=bk_s, in_=bk_d.rearrange("(m p) -> p m", p=128))
            nc.sync.dma_start(out=wq, in_=wq_t)
            hsT8_t = hsT8_d.rearrange("(t p) s -> p t s", p=128)
            for k in range(KT):
                e = nc.sync if k % 2 == 0 else nc.gpsimd
                e.dma_start(out=hsT8[:, k, 0:512], in_=hsT8_t[:, k, 0:512])
            nc.gpsimd.dma_start(out=wk, in_=wk_t)
            nc.gpsimd.dma_start(out=bv_b, in_=bass.AP(
                tensor=bv_d[:].tensor, offset=0, ap=[[0, 128], [1, DC]]))
            nc.sync.dma_start(
                out=wv8, in_=wvT8_d.rearrange("(t p) c -> p t c", p=128))
            for k in range(KT):
                e = nc.sync if k % 2 == 0 else nc.gpsimd
                e.dma_start(out=hsT8[:, k, 512:2048], in_=hsT8_t[:, k, 512:2048])
            for k in range(KT):
                e = nc.sync if k % 2 == 0 else nc.gpsimd
                e.dma_start(out=hsT[:, k, 0:1024], in_=hsT_t[:, k, 0:1024])
            nc.gpsimd.dma_start(out=wp, in_=wp_t)
            for k in range(KT):
                e = nc.sync if k % 2 == 0 else nc.gpsimd
                e.dma_start(out=hsT[:, k, 1024:2048],
                            in_=hsT_t[:, k, 1024:2048])
            nc.gpsimd.dma_start(out=gm_b, in_=bass.AP(
                tensor=gm_d[:].tensor, offset=0, ap=[[0, 128], [1, DC]]))
            nc.gpsimd.dma_start(out=bt_b, in_=bass.AP(
                tensor=bt_d[:].tensor, offset=0, ap=[[0, 128], [1, DC]]))
            nc.vector.memset(onesr, 1.0)
            nc.vector.memset(eps_c, EPS)
            nc.vector.memset(wsrc, 1.0)
            nc.vector.memset(vA[:, :, :, :, HD:MP], 0.0)
            nc.vector.memset(vA[:, :, :, :, HD:HD + 1], 1.0)

            with (
                tc.tile_pool(name="pps", bufs=2, space="PSUM") as pps,
                tc.tile_pool(name="scps", bufs=2, space="PSUM") as scps,
                tc.tile_pool(name="ctxps", bufs=2, space="PSUM") as ctxps,
                tc.tile_pool(name="ptp", bufs=4) as ptp,
                tc.tile_pool(name="small", bufs=2) as small,
                tc.tile_pool(name="stg", bufs=2) as stg,
                tc.tile_pool(name="xrp", bufs=2) as xrp,
                tc.tile_pool(name="rows", bufs=2) as rows,
            ):
                # PE warm-up: ~28 cheap matmuls so pe ramps to full clock
                # before the first real projection (which waits on DMAs).
                wps = pps.tile([128, 512], F32, name="gps")
                for i in range(24):
                    nc.tensor.matmul(out=wps[0:1, 0:256], lhsT=onesr,
                                     rhs=wsrc, start=True, stop=True)

                def proj_group(w_sb, m, n, bias, out_sb):
                    """One [128,512] q/k projection block, fp8 DoubleRow
                    (scores are softmax-diluted; fp8 error is negligible
                    downstream)."""
                    ps = pps.tile([128, 512], F32, name="gps")
                    for t in range(KT // 2):
                        nc.tensor.matmul(
                            out=ps,
                            lhsT=w_sb[:, 2 * t:2 * t + 2, m, :],
                            rhs=hsT8[:, 2 * t:2 * t + 2,
                                     n * 512:(n + 1) * 512],
                            start=(t == 0), stop=(t == KT // 2 - 1),
                            perf_mode=DR)
                    o = out_sb[:, m, n * 512:(n + 1) * 512]
                    nc.vector.tensor_scalar_add(out=o, in0=ps, scalar1=bias)

                def v_group(j):
                    """V for key tile j via fp8 DoubleRow (4x fewer PE
                    cycles; V is quantized to fp8e4 downstream anyway)."""
                    kp, par = divmod(j, 2)
                    ps = pps.tile([128, 512], F32, name="gps")
                    for t in range(KT // 2):
                        nc.tensor.matmul(
                            out=ps[:, 0:DC],
                            lhsT=hsT8[:, 2 * t:2 * t + 2,
                                      j * 128:(j + 1) * 128],
                            rhs=wv8[:, 2 * t:2 * t + 2, :],
                            start=(t == 0), stop=(t == KT // 2 - 1),
                            perf_mode=DR)
                    nc.vector.tensor_add(
                        out=vA[:, kp, par, :, 0:HD],
                        in0=ps[:, 0:DC].rearrange("p (h d) -> p h d", d=HD),
                        in1=bv_b.rearrange("p (h d) -> p h d", d=HD))

                def r_group(qt):
                    """Residual hs@Wp.T for query tile qt, natural layout,
                    straight into q-major x."""
                    ps = pps.tile([128, 512], F32, name="gps")
                    for k in range(KT):
                        nc.tensor.matmul(
                            out=ps[:, 0:DC],
                            lhsT=hsT[:, k, qt * 128:(qt + 1) * 128],
                            rhs=wp[:, k, :],
                            start=(k == 0), stop=(k == KT - 1))
                    nc.vector.tensor_copy(out=x[:, qt, :], in_=ps[:, 0:DC])

                def g_q(m, n):
                    return lambda: proj_group(wq, m, n, bq_s[:, m:m + 1], qT)

                def g_k(m, n):
                    return lambda: proj_group(wk, m, n, bk_s[:, m:m + 1], kT)

                def g_r(qt):
                    return lambda: r_group(qt)

                for g in [g_q(0, 0), g_k(0, 0)]:
                    g()
                blk0 = {1: g_k(0, 1), 3: g_k(0, 2), 5: g_k(0, 3),
                        7: g_q(0, 1), 9: g_r(0), 11: g_r(1), 13: g_r(2),
                        15: g_r(3)}
                fillers = [
                    g_q(0, 2), g_r(4), g_r(5), g_r(6), g_r(7), g_q(0, 3),
                    g_k(1, 0), g_k(1, 1),
                    g_k(1, 2), g_k(1, 3), g_q(1, 0), g_r(8), g_r(9),
                    g_r(10), g_r(11), g_q(1, 1),
                    g_q(1, 2), g_q(1, 3), g_r(12), g_r(13), g_r(14),
                    g_r(15),
                ]
                fillers.reverse()   # consumed via pop()

                cc_in = dram.tile([NS, 1024], F32)
                cc_out = dram.tile([NS, GROUPS * 1024], F32)
                cc_in23 = dram.tile([1, 2048], F32)   # qp-major: qp*16+(qn-2)*8+v*4+qt
                cc_out23 = dram.tile([1, GROUPS * 2048], F32)
                out_t = out_d.rearrange("(t p) c -> p t c", p=128)

                # ---------------- pipeline stages ----------------
                def s1a_xupdate(hp, qn, ctxc, den_r):
                    """x[q, head dims] += ctx * recip(denom); per-partition
                    scalars only. Deferred one block; pure DVE."""
                    for h2 in range(2):
                        g = 2 * hp + h2
                        for qt in range(4):
                            tmp = stg.tile([128, HD], F32, name="tmp")
                            nc.vector.tensor_scalar_mul(
                                out=tmp,
                                in0=ctxc[:, h2, qt * 128:qt * 128 + HD],
                                scalar1=den_r[:, h2, qt:qt + 1])
                            xs = x[:, 4 * qn + qt, g * HD:(g + 1) * HD]
                            nc.vector.tensor_tensor(
                                out=xs, in0=xs, in1=tmp, op=ALU.add)
                    if hp != MT - 1:
                        return None
                    xq = x[:, 4 * qn:4 * qn + 4, :]
                    xr = xrp.tile([128, 4, DC], F32, name="xr")
                    nc.vector.tensor_scalar_max(out=xr, in0=xq, scalar1=0.0)
                    x2 = xrp.tile([128, 4, DC], F32, name="x2")
                    nc.vector.tensor_tensor(
                        out=x2, in0=xr, in1=xr, op=ALU.mult)
                    return xr, x2

                def s1b_stats(qn, xr, x2, do_ag=True):
                    """local stats via free-dim reduces + AllGather issue.
                    Chunks 2 and 3 share one merged AllGather at the drain
                    (the two would otherwise serialize on the collective
                    device right at the end)."""
                    st = rows.tile([128, 2, 4], F32, name="st")
                    nc.vector.tensor_reduce(
                        out=st[:, 0, :], in_=xr, axis=mybir.AxisListType.X,
                        op=ALU.add)
                    nc.vector.tensor_reduce(
                        out=st[:, 1, :], in_=x2, axis=mybir.AxisListType.X,
                        op=ALU.add)
                    if not do_ag:
                        # qp-major staging for the merged chunk-2/3 gather
                        nc.sync.dma_start(
                            out=bass.AP(tensor=cc_in23.tensor,
                                        offset=cc_in23.offset + (qn - 2) * 8,
                                        ap=[[16, 128], [1, 8]]),
                            in_=st)
                        return
                    # dram layout per rank: [kind v][q = qt*128 + qp]
                    nc.sync.dma_start(
                        out=bass.AP(tensor=cc_in.tensor,
                                    offset=cc_in.offset + qn * 1024,
                                    ap=[[1, 128], [512, 2], [128, 4]]),
                        in_=st)
                    if do_ag:
                        nc.gpsimd.collective_compute(
                            "AllGather", ALU.bypass,
                            replica_groups=[[0, 1, 2, 3], [4, 5, 6, 7]],
                            ins=[cc_in[qn:qn + 1, :].opt()],
                            outs=[cc_out[qn:qn + 1, :].opt()],
                        )

                def s2_rowmath(qn, dmae=None, merged=False):
                    """group stats -> per-query A (rstd), B (mean*rstd)."""
                    dmae = dmae or nc.sync
                    if merged:
                        # rank stride 2048 != contiguous -> keep 3-dim APs
                        # on both sides (pad dest stride to 9 so it can't
                        # auto-merge)
                        rsb = rows.tile([128, GROUPS, 8], F32, name="rsb")
                        src_ap = bass.AP(
                            tensor=cc_out23.tensor,
                            offset=cc_out23.offset + (qn - 2) * 8,
                            ap=[[16, 128], [2048, GROUPS], [1, 8]])
                        dmae.dma_start(out=rsb, in_=src_ap)
                        rs = [rsb[:, r] for r in range(GROUPS)]
                    else:
                        rsb = rows.tile([128, GROUPS, 2, 4], F32, name="rsb")
                        src_ap = bass.AP(
                            tensor=cc_out.tensor,
                            offset=cc_out.offset + qn * GROUPS * 1024,
                            ap=[[1, 128], [1024, GROUPS], [512, 2],
                                [128, 4]])
                        dmae.dma_start(out=rsb, in_=src_ap)
                        rs = [rsb[:, r].rearrange("p a b -> p (a b)")
                              for r in range(GROUPS)]
                    acc = rows.tile([128, 8], F32, name="acc")
                    nc.vector.tensor_tensor(
                        out=acc, in0=rs[0], in1=rs[1], op=ALU.add)
                    nc.vector.tensor_tensor(
                        out=acc, in0=acc, in1=rs[2], op=ALU.add)
                    nc.vector.tensor_tensor(
                        out=acc, in0=acc, in1=rs[3], op=ALU.add)
                    mm = rows.tile([128, 8], F32, name="mm")
                    nc.vector.tensor_scalar_mul(
                        out=mm, in0=acc, scalar1=1.0 / D)
                    var = rows.tile([128, 4], F32, name="var")
                    nc.vector.tensor_tensor(
                        out=var, in0=mm[:, 0:4], in1=mm[:, 0:4],
                        op=ALU.mult)
                    nc.vector.tensor_tensor(
                        out=var, in0=mm[:, 4:8], in1=var, op=ALU.subtract)
                    sd = rows.tile([128, 4], F32, name="sd")
                    nc.scalar.activation(
                        out=sd, in_=var, func=AF.Sqrt, bias=eps_c)
                    AB = rows.tile([128, 2, 4], F32, name="AB")
                    nc.vector.reciprocal(out=AB[:, 0, :], in_=sd)
                    nc.vector.tensor_tensor(
                        out=AB[:, 1, :], in0=mm[:, 0:4], in1=AB[:, 0, :],
                        op=ALU.mult)
                    return AB

                def s3_apply(qn, AB, dmae=None, pool=False):
                    """out = (relu(x)*A - B)*gamma + beta, DMA out."""
                    dmae = dmae or nc.sync
                    for qt4 in range(4):
                        qt = 4 * qn + qt4
                        eng = nc.gpsimd if (pool and qt4 % 2) else nc.vector
                        y = stg.tile([128, DC], F32, name="y")
                        eng.tensor_scalar(
                            out=y, in0=x[:, qt, :],
                            scalar1=0.0, scalar2=AB[:, 0, qt4:qt4 + 1],
                            op0=ALU.max, op1=ALU.mult)
                        eng.tensor_scalar(
                            out=y, in0=y, scalar1=AB[:, 1, qt4:qt4 + 1],
                            scalar2=None, op0=ALU.subtract)
                        eng.tensor_tensor(
                            out=y, in0=y, in1=gm_b, op=ALU.mult)
                        eng.tensor_tensor(
                            out=x[:, qt, :], in0=y, in1=bt_b, op=ALU.add)
                        if qt4 == 1:
                            dmae.dma_start(
                                out=out_t[:, 4 * qn:4 * qn + 2, :],
                                in_=x[:, 4 * qn:4 * qn + 2, :])
                    dmae.dma_start(
                        out=out_t[:, 4 * qn + 2:4 * qn + 4, :],
                        in_=x[:, 4 * qn + 2:4 * qn + 4, :])

                pend_xu = []      # (hp, qn, ctxc, den_r)
                pend_st = []      # (qn, xr, x2)
                pend_ag = []      # (qn, issue_block)
                pend_s3 = []      # (qn, AB)
                bi = 0

                def do_s1a():
                    if pend_xu:
                        hp_, qn_, ctxc_, den_ = pend_xu.pop(0)
                        r = s1a_xupdate(hp_, qn_, ctxc_, den_)
                        if r is not None:
                            pend_st.append((qn_, r[0], r[1]))

                def do_s1b(bi):
                    if pend_st:
                        qn_, xr_, x2_ = pend_st.pop(0)
                        s1b_stats(qn_, xr_, x2_, do_ag=True)
                        pend_ag.append((qn_, bi))

                def do_s2(bi, min_age=1, dmae=None):
                    if pend_ag and bi - pend_ag[0][1] >= min_age:
                        qn_, _ = pend_ag.pop(0)
                        pend_s3.append((qn_, s2_rowmath(qn_, dmae)))

                # ================= attention =================
                BLOCKS = [(0, 0), (0, 1), (0, 2), (0, 3),
                          (1, 0), (1, 1), (1, 2), (1, 3)]
                if True:
                    for hp, qn in BLOCKS:
                        qs = slice(qn * 512, (qn + 1) * 512)
                        # one PSUM bank hosts 4 accumulation groups (one
                        # per query tile): matmul start=True zeroing is
                        # bank-granular, so pre-zero via DVE and accumulate
                        # with start=False throughout.
                        ctxA = ctxps.tile([128, 512], F32, name="ctx")
                        ctxB = ctxps.tile([128, 512], F32, name="ctx")
                        nc.vector.memset(ctxA, 0.0)
                        nc.vector.memset(ctxB, 0.0)

                        def ctx_mms(pt, kp):
                            for h2, cps in ((0, ctxA), (1, ctxB)):
                                for qt4 in range(4):
                                    nc.tensor.matmul(
                                        out=cps[:, qt4 * 128:qt4 * 128 + HD + 1],
                                        lhsT=pt[:, :, h2,
                                                qt4 * 128:(qt4 + 1) * 128],
                                        rhs=vA[:, kp, :, 2 * hp + h2,
                                               0:HD + 1],
                                        start=False, stop=(kp == KP - 1),
                                        perf_mode=DR)

                        prev = None
                        for kp in range(KP):
                            pt = ptp.tile([128, 2, 2, 512], F8E5, name="pt")
                            for par in range(2):
                                ks = 2 * kp + par
                                sc = scps.tile([128, 1024], F32, name="sc")
                                kslc = slice(ks * 128, (ks + 1) * 128)
                                nc.tensor.matmul(
                                    out=sc[:, 0:512],
                                    lhsT=kT[0:64, hp, kslc],
                                    rhs=qT[0:64, hp, qs])
                                nc.tensor.matmul(
                                    out=sc[:, 512:1024],
                                    lhsT=kT[64:128, hp, kslc],
                                    rhs=qT[64:128, hp, qs])
                                nc.scalar.activation(
                                    out=pt[:, par], in_=sc, func=AF.Exp,
                                    scale=float(1.0 / np.sqrt(HD)))
                                if hp == 0 and qn == 0:
                                    v_group(ks)
                                    if ks in blk0:
                                        blk0[ks]()
                                elif fillers and ks % 2 == 0:
                                    fillers.pop()()
                            if prev is not None:
                                ctx_mms(*prev)
                                if kp == 1:
                                    do_s1a()
                                elif kp == 2:
                                    do_s1b(bi)
                                elif kp == 4:
                                    do_s2(bi)
                            prev = (pt, kp)
                        ctx_mms(*prev)

                        # denominators (per-partition!) + ctx copy-out
                        den_r = small.tile([128, 2, 4], F32, name="den")
                        for h2, cps in ((0, ctxA), (1, ctxB)):
                            nc.vector.reciprocal(
                                out=den_r[:, h2, :],
                                in_=bass.AP(tensor=cps.tensor,
                                            offset=cps.offset + HD,
                                            ap=[list(cps.ap[0]), [128, 4]]))
                        ctxc = stg.tile([128, 2, 512], F32, name="ctxc")
                        nc.vector.tensor_copy(out=ctxc[:, 0, :], in_=ctxA)
                        nc.vector.tensor_copy(out=ctxc[:, 1, :], in_=ctxB)
                        pend_xu.append((hp, qn, ctxc, den_r))
                        if pend_s3:
                            s3_apply(*pend_s3.pop(0))
                        bi += 1

                # drain
                do_s1a()
                do_s1b(bi)
                do_s2(bi, min_age=0)                   # qn=2
                do_s2(bi, min_age=0, dmae=nc.scalar)   # qn=3 (waits AG(3))
                s3_apply(*pend_s3.pop(0), pool=True)   # qn=2
                s3_apply(*pend_s3.pop(0), dmae=nc.scalar, pool=True)
            p1sb_cm.__exit__(None, None, None)
    _split_waits(nc)
    return nc


_NC = None
LAST_RESULT = None


def _get_nc():
    global _NC
    if _NC is None:
        _NC = build_bass()
    return _NC


def kernel(hidden_states, Wq, bq, Wk, bk, Wv, bv, Wp, gamma, beta):
    hs = np.asarray(hidden_states, dtype=np.float32)
    Wq = np.asarray(Wq, np.float32)
    Wk = np.asarray(Wk, np.float32)
    Wv = np.asarray(Wv, np.float32)
    Wp = np.asarray(Wp, np.float32)
    bq = np.asarray(bq, np.float32)
    bk = np.asarray(bk, np.float32)
    bv = np.asarray(bv, np.float32)
    gamma = np.asarray(gamma, np.float32)
    beta = np.asarray(beta, np.float32)
    bf = ml_dtypes.bfloat16

    nc = _get_nc()
    in_maps = []
    for c in range(NCORES):
        b, g = divmod(c, GROUPS)
        sl = slice(g * DC, (g + 1) * DC)
        in_maps.append({
            "hsT": np.ascontiguousarray(hs[b].T.astype(bf)),
            "hsT8": np.ascontiguousarray(
                hs[b].T.astype(ml_dtypes.float8_e4m3)).view(np.uint8),
            "wvT8": np.ascontiguousarray(
                Wv[sl].T.astype(ml_dtypes.float8_e4m3)).view(np.uint8),
            "wqT": np.ascontiguousarray(
                Wq[sl].T.astype(ml_dtypes.float8_e4m3)).view(np.uint8),
            "wkT": np.ascontiguousarray(
                Wk[sl].T.astype(ml_dtypes.float8_e4m3)).view(np.uint8),
            "wpT": np.ascontiguousarray(Wp[sl].T.astype(bf)),
            "bq": np.ascontiguousarray(bq[sl]),
            "bk": np.ascontiguousarray(bk[sl]),
            "bv": np.ascontiguousarray(bv[sl]),
            "gamma": np.ascontiguousarray(gamma[sl]),
            "beta": np.ascontiguousarray(beta[sl]),
        })
    res = run_bass_kernel_spmd(nc, in_maps, core_ids=list(range(NCORES)))
    global LAST_RESULT
    LAST_RESULT = res
    out = np.empty((B, S, D), np.float32)
    for c, r in enumerate(res.results):
        b, g = divmod(c, GROUPS)
        out[b, :, g * DC:(g + 1) * DC] = r["out"]
    return out
